# revision 29
# baseline (speedup 1.0000x reference)
"""Llama3 GQA causal attention (B=1, T=2048, D=4096, 32 Q heads / 8 KV heads,
dh=128) on 8 Trainium2 NeuronCores.

Sharding: tensor-parallel over heads. Core i owns KV head i and Q heads
4i..4i+3: Wq/Wk/Wv split column-wise, Wo split row-wise. Each core computes a
partial [T, D] output (rows of Wo for its heads); the host sums the 8 partials.

Device layout notes:
 - resid is transposed on the host to rT [D, T] so every projection matmul has
   its contraction dim (d) on partitions with no on-device transpose.
 - Q/K are produced transposed (Q^T [dh, T]) which is exactly the layout the
   scores matmul wants; scores are computed transposed (S^T [Tk, Tq]) so the
   softmax denominator comes from an all-ones-matrix matmul (which also
   broadcasts it to all 128 partitions) and probabilities can be consumed
   directly by the ctx matmul (ctx^T = V^T @ P^T) with V stationary.
 - everything runs in fp16 (fp32 PSUM accumulation): same PE rate as bf16 but
   8x the mantissa, 4x DVE element-wise rate on SBUF tiles, and fp16 output
   partials halve the output DMA.
 - scores matmuls pack two 512-wide fp32 tiles into one 2-bank [128, 1024]
   PSUM region, so the scalar engine runs ONE exp ACTIVATE per two tiles
   (amortizing the ~230ns per-ACTIVATE overhead).
 - the attention phases are scalar-exp-throughput-bound while the Wo phases
   leave the scalar engine ~70% idle, so attention chunks 2 and 3 are
   EMISSION-INTERLEAVED into Wo chunks 0 and 1: the PE alternates Wo blocks
   and attention units, and the attention exps run against the Wo phase's
   idle scalar time. Attention is pipelined globally across heads (not per
   head), so one head's exp latency is hidden by the next head's scores.
 - the softmax denominator uses an all-ones [128,128] stationary, which both
   sums over keys and broadcasts the result to every partition in the same
   matmul; the reciprocal is computed as exp(-ln(den)) on the scalar engine
   with a manually preloaded natural_log_exp_and_others activation table set
   (covers Exp AND Ln), so the whole kernel needs exactly one
   ACT_TABLE_LOAD. The ln/exp/scale run deferred inside the NEXT phase's
   scalar-idle window.
 - causal structure is exploited at 128-column granularity: the four
   diagonal-region tiles (F = 512-128r) pack into two pair regions; only the
   leading 128 columns of each need the triangular mask (gpsimd).
 - a short warm-up burst of dummy matmuls runs during the ~10us startup DMA
   window so the PE's HAM clock gate is already at 2.4 GHz (not the cold
   1.2 GHz) when the first real matmul issues.
PSUM budget: tag "psq" = 2 bufs x 2 banks (proj q-accs / score pairs / wo is
not using it), tag "ps" = 4 bufs x 1 bank (proj k/v accs, V transposes,
ctx/den, wo blocks). Total exactly 8 banks.
"""

import math
import sys

import numpy as np

sys.path.insert(0, "/opt/trn_rl_repo")

import bass_rust

import concourse.bass as bass
import concourse.mybir as mybir
import concourse.tile as tile
from concourse.bass_utils import run_bass_kernel_spmd
from concourse.hw_specs import get_activation_tables

F16 = mybir.dt.float16
F32 = mybir.dt.float32
ACT_COPY = mybir.ActivationFunctionType.Copy
ACT_EXP = mybir.ActivationFunctionType.Exp
ACT_LN = mybir.ActivationFunctionType.Ln

D_MODEL = 4096
N_HEADS = 32
N_KV = 8
DH = 128
T = 2048
NCORES = 8
HQ = N_HEADS // NCORES  # 4 q heads per core
NT = T // 128  # 16 row tiles
NCH = T // 512  # 4 column chunks
SCALE = 1.0 / math.sqrt(DH)
# softmax bias: p = exp(s*SCALE - EXP_BIAS). Cancels between numerator and
# denominator; keeps exp() inside fp16 range.
EXP_BIAS = -3.5
ROPE = dict(
    rope_theta=500000.0,
    factor=32.0,
    hi_freq_factor=4.0,
    lo_freq_factor=1.0,
    original_context_length=8192,
)


def _rope_tables():
    """cos/sin tables in transposed layout [dh, T]; sin has the rotate-half
    sign folded in (rows 0:64 negated)."""
    idx = np.arange(0, DH, 2, dtype=np.float64) / DH
    freq = (1.0 / (2.0 * math.pi)) * ROPE["rope_theta"] ** (-idx)
    factor, lo, hi = ROPE["factor"], ROPE["lo_freq_factor"], ROPE["hi_freq_factor"]
    L0 = ROPE["original_context_length"]
    freq_low, freq_high = lo / L0, hi / L0
    freq_scaled = np.where(freq < freq_low, freq / factor, freq)
    smooth = np.clip((L0 * freq - lo) / (hi - lo), 0.0, 1.0)
    freq_smooth = (1.0 - smooth) * (freq / factor) + smooth * freq
    is_mid = (freq >= freq_low) & (freq <= freq_high)
    freq = np.where(is_mid, freq_smooth, freq_scaled)
    pos = np.arange(T, dtype=np.float64)
    phase = 2.0 * math.pi * pos[:, None] * freq[None, :]  # [T, 64]
    emb = np.concatenate([phase, phase], axis=-1)  # [T, 128]
    cos = np.cos(emb)
    sin = np.sin(emb)
    cosT = np.ascontiguousarray(cos.T).astype(np.float16)  # [128, T]
    sinT = np.ascontiguousarray(sin.T)
    sinM = sinT.copy()
    sinM[:64] = -sinT[:64]
    return cosT, sinM.astype(np.float16)


def _build_nc():
    nc = bass.Bass()
    rt = nc.dram_tensor("rt", [D_MODEL, T], F16, kind="ExternalInput")
    wq = nc.dram_tensor("wq", [D_MODEL, HQ * DH], F16, kind="ExternalInput")
    wk = nc.dram_tensor("wk", [D_MODEL, DH], F16, kind="ExternalInput")
    wv = nc.dram_tensor("wv", [D_MODEL, DH], F16, kind="ExternalInput")
    wo = nc.dram_tensor("wo", [HQ * DH, D_MODEL], F16, kind="ExternalInput")
    cosT = nc.dram_tensor("cosT", [DH, T], F16, kind="ExternalInput")
    sinM = nc.dram_tensor("sinM", [DH, T], F16, kind="ExternalInput")
    tri = nc.dram_tensor("tri", [128, 128], F16, kind="ExternalInput")
    outp = nc.dram_tensor("outp", [T, D_MODEL], F16, kind="ExternalOutput")

    rt3 = rt.rearrange("(o p) t -> p o t", p=128)  # [128, 32, T]
    wq3 = wq.rearrange("(o p) m -> p o m", p=128)  # [128, 32, 512]
    wk3 = wk.rearrange("(o p) m -> p o m", p=128)  # [128, 32, 128]
    wv3 = wv.rearrange("(o p) m -> p o m", p=128)
    wo3 = wo.rearrange("(o p) n -> p o n", p=128)  # [128, 4, 4096]

    with tile.TileContext(nc) as tc:
        with (
            tc.tile_pool(name="consts", bufs=1) as cpool,
            tc.tile_pool(name="acts", bufs=1) as apool,
            tc.tile_pool(name="rtp", bufs=8) as rpool,
            tc.tile_pool(name="scr", bufs=2) as spool,
            tc.tile_pool(name="pt", bufs=4) as ppool,
            tc.tile_pool(name="ob", bufs=2) as opool,
            tc.tile_pool(name="ps", bufs=1, space="PSUM") as ps,
        ):
            tri_sb = cpool.tile([128, 128], F16)

            def load_consts():
                nc.sync.dma_start(tri_sb, tri[:, :])
            ones_sb = cpool.tile([128, 128], F16)
            nc.gpsimd.memset(ones_sb, 1.0)
            ebias_sb = cpool.tile([128, 1], F32)
            nc.gpsimd.memset(ebias_sb, EXP_BIAS)
            # warm-up operand comes from a tiny DMA (the sync engine is ready
            # ~2us before gpsimd finishes its preamble memsets)
            warm_sb = cpool.tile([128, 128], F16)
            nc.sync.dma_start(warm_sb, tri[:, :])
            # preload the one activation-table set covering every function
            # this kernel uses (Exp, Ln, Copy); the bacc fixpoint pass then
            # inserts no further ACT_TABLE_LOADs.
            combo_id = list(get_activation_tables(nc.m.arch)).index(
                "natural_log_exp_and_others"
            )
            ld = mybir.InstLoadActFuncSet(
                name=nc.get_next_instruction_name(), ins=[], outs=[]
            )
            ld.act_func_set_id = combo_id
            nc.scalar.add_instruction(ld)
            # HAM warm-up: ~3.9us of short dummy matmuls during the startup
            # DMA window. They un-throttle the PE clock gate (1.2 -> 2.4 GHz)
            # before the first real matmul, and end before the first real
            # matmul's inputs have landed.
            warm_ps = ps.tile([128, 512], F32, tag="ps", bufs=4, name="warm")
            for _ in range(36):
                nc.tensor.matmul(
                    warm_ps[:, 0:128], warm_sb, warm_sb, start=True, stop=True
                )

            cos_sb = cpool.tile([DH, T], F16)
            sin_sb = cpool.tile([DH, T], F16)
            wq_sb = cpool.tile([128, 32, HQ * DH], F16)
            wk_sb = cpool.tile([128, 32, DH], F16)
            wv_sb = cpool.tile([128, 32, DH], F16)
            wo_sb = cpool.tile([128, HQ, D_MODEL], F16)

            # activations that persist across phases
            qt_sb = apool.tile([128, HQ, T], F16)  # Q^T per head, rope'd
            kt_sb = apool.tile([128, T], F16)  # K^T, rope'd
            v_sb = apool.tile([128, NT, DH], F16)  # V tiles [tk, j, dh]
            cx_sb = apool.tile([128, HQ, T], F16)  # normalized ctx^T

            # deferred normalization: (c, h, cxu, den16); ln/exp/scale all
            # run at flush time, inside a later phase's scalar-idle window.
            pend = []
            # deferred RoPE tiles, flushed a few per phase.
            rope_pend = []

            def flush_rope(n=1):
                for _ in range(min(n, len(rope_pend))):
                    rope_pend.pop(0)()

            def finish_norm():
                if not pend:
                    return
                c, h, cxu, den16 = pend.pop(0)
                cs = slice(512 * c, 512 * (c + 1))
                # rec = exp(-ln(den)); Ln and Exp share the preloaded table
                # set, so no ACT_TABLE_LOADs are triggered.
                nc.scalar.activation(den16, den16, ACT_LN)
                rec16 = spool.tile(
                    [128, 512], F16, tag="rec16", bufs=2, name=f"r16_{c}_{h}"
                )
                nc.scalar.activation(rec16, den16, ACT_EXP, scale=-1.0)
                # all-SBUF fp16 multiply on the otherwise-idle gpsimd engine
                nc.gpsimd.tensor_mul(cx_sb[:, h, cs], cxu, rec16)

            def proj_chunk(c):
                cs = slice(512 * c, 512 * (c + 1))
                # q accumulators pair-packed into two 2-bank PSUM tiles;
                # k and v accumulators in single-bank tiles
                aq = [
                    ps.tile([128, 1024], F32, tag="psq", bufs=2, name=f"acc{c}_{i}")
                    for i in range(2)
                ]
                ak = ps.tile([128, 512], F32, tag="ps", bufs=4, name=f"acck{c}")
                av = ps.tile([128, 512], F32, tag="ps", bufs=4, name=f"accv{c}")

                def acc(i):
                    if i < 4:
                        return aq[i // 2][:, 512 * (i % 2) : 512 * (i % 2 + 1)]
                    return ak if i == 4 else av

                for o2 in range(16):
                    rtt2 = rpool.tile([128, 2, 512], F16, tag="rt", bufs=6)
                    if c == 0 and o2 == 0:
                        # singles: first matmul's dependency is ~256KB of
                        # DMA, not ~1MB
                        nc.sync.dma_start(rtt2[:, 0, :], rt3[:, 0, cs])
                        nc.sync.dma_start(wq_sb[:, 0, :], wq3[:, 0, :])
                        nc.sync.dma_start(wk_sb[:, 0:1, :], wk3[:, 0:1, :])
                        nc.sync.dma_start(wv_sb[:, 0:1, :], wv3[:, 0:1, :])
                        nc.sync.dma_start(rtt2[:, 1, :], rt3[:, 1, cs])
                        nc.sync.dma_start(wq_sb[:, 1, :], wq3[:, 1, :])
                        nc.sync.dma_start(wk_sb[:, 1:4, :], wk3[:, 1:4, :])
                        nc.sync.dma_start(wv_sb[:, 1:4, :], wv3[:, 1:4, :])
                    else:
                        nc.sync.dma_start(rtt2, rt3[:, 2 * o2 : 2 * o2 + 2, cs])
                        if c == 0 and o2 == 12:
                            # rope tables + mask, issued late in chunk 0 where
                            # the weight streams have finished: off both the
                            # chunk-0 and the chunk-1 rt critical paths
                            load_consts()
                            nc.sync.dma_start(cos_sb, cosT[:, :])
                            nc.sync.dma_start(sin_sb, sinM[:, :])
                        if c == 0:
                            o = 2 * o2
                            nc.sync.dma_start(
                                wq_sb[:, o : o + 2, :], wq3[:, o : o + 2, :]
                            )
                            if o % 4 == 0:
                                nc.sync.dma_start(
                                    wk_sb[:, o : o + 4, :], wk3[:, o : o + 4, :]
                                )
                                nc.sync.dma_start(
                                    wv_sb[:, o : o + 4, :], wv3[:, o : o + 4, :]
                                )
                    for oo in range(2):
                        o = 2 * o2 + oo
                        rtt = rtt2[:, oo, :]
                        st, sp = (o == 0), (o == 31)
                        for h in range(HQ):
                            nc.tensor.matmul(
                                acc(h), wq_sb[:, o, 128 * h : 128 * (h + 1)], rtt,
                                start=st, stop=sp,
                            )
                        nc.tensor.matmul(
                            acc(4), wk_sb[:, o, :], rtt, start=st, stop=sp
                        )
                        nc.tensor.matmul(
                            acc(5), wv_sb[:, o, :], rtt, start=st, stop=sp
                        )
                        if o in (8, 12, 16, 20):
                            finish_norm()  # previous attn chunk's norms
                        if o in (6, 10, 14, 18, 22):
                            flush_rope()  # pending rope tiles
                        if c == 2 and o in (8, 14, 20, 26):
                            hh = (o - 8) // 6
                            nc.sync.dma_start(wo_sb[:, hh, :], wo3[:, hh, :])
                # drain PSUM fast: one cast per accumulator (split
                # scalar/vector), then rope runs on fp16 SBUF tiles
                xq = []
                for idx in range(5):
                    x = spool.tile([128, 512], F16, tag=f"x{idx}")
                    if idx == 0:
                        nc.scalar.activation(x, acc(idx), ACT_COPY)
                    else:
                        nc.vector.tensor_copy(x, acc(idx))
                    xq.append(x)
                vt = spool.tile([128, 512], F16, tag="vt")

                def rope_tile(idx, x=None):
                    def go(x=x):
                        xs = spool.tile([128, 512], F16, tag="xs", name=f"xs{c}_{idx}")
                        nc.vector.tensor_copy(xs[0:64, :], x[64:128, :])
                        nc.vector.tensor_copy(xs[64:128, :], x[0:64, :])
                        t1 = spool.tile([128, 512], F16, tag="t1", name=f"t1{c}_{idx}")
                        nc.vector.tensor_mul(t1, x, cos_sb[:, cs])
                        nc.vector.tensor_mul(xs, xs, sin_sb[:, cs])
                        dst = qt_sb[:, idx, cs] if idx < HQ else kt_sb[:, cs]
                        nc.vector.tensor_add(dst, t1, xs)
                    return go

                for idx in range(5):
                    rope_pend.append(rope_tile(idx, xq[idx]))
                # drain the v accumulator (split scalar/vector, frees its
                # PSUM bank), then V^T -> V via DMA-xbar transposes on the
                # otherwise-idle DMA engines. No PE instruction sits at the
                # phase boundary, so the next phase's matmuls never stall
                # behind the vt drain.
                nc.scalar.activation(vt[:, 0:256], acc(5)[:, 0:256], ACT_COPY)
                nc.vector.tensor_copy(vt[:, 256:512], acc(5)[:, 256:512])
                for s in range(4):
                    nc.sync.dma_start_transpose(
                        v_sb[:, 4 * c + s, :], vt[:, 128 * s : 128 * (s + 1)]
                    )

            def attn_units(c, lagp, ps_diag=False):
                """Incremental emitter for attention chunk c: a generator
                yielding after each (scores+exp | ctx+den) pipeline step.
                Globally pipelined across heads: unit stream is
                [(h,t) for h in heads for t in units-of-head]."""
                npair = 2 * c  # full pairs per head
                nunits = npair + 2  # + two diagonal pairs
                nj = 4 * (c + 1)
                state = {}  # h -> (ctx_ps, den_ps)
                p_tiles = {}
                pa_tiles = {}

                def emit_scores(h, t):
                    # standalone chunk 0 has no proj competing for "ps", so
                    # its second (narrow) diagonal pair allocates there --
                    # doubling the score-buffer depth of the exp pipeline.
                    if ps_diag and t == npair + 1:
                        s_ps = ps.tile(
                            [128, 512], F32, tag="ps", bufs=4, name=f"s{c}_{h}_{t}"
                        )
                    else:
                        s_ps = ps.tile(
                            [128, 1024], F32, tag="psq", bufs=2, name=f"s{c}_{h}_{t}"
                        )
                    p = ppool.tile([128, 1024], F16, tag="pt", name=f"p{c}_{h}_{t}")
                    if t < npair:  # full pair
                        qs_full = qt_sb[:, h, 512 * c : 512 * (c + 1)]
                        for u in range(2):
                            j = 2 * t + u
                            nc.tensor.matmul(
                                s_ps[:, 512 * u : 512 * (u + 1)],
                                kt_sb[:, 128 * j : 128 * (j + 1)],
                                qs_full,
                                start=True,
                                stop=True,
                            )
                        nc.scalar.activation(
                            p, s_ps, ACT_EXP, bias=ebias_sb, scale=SCALE
                        )
                        pa = spool.tile(
                            [128, 512], F16, tag="pa", bufs=4, name=f"pa{c}_{h}_{t}"
                        )
                        nc.vector.tensor_add(pa, p[:, 0:512], p[:, 512:1024])
                        if t % 2 == 1:
                            paq = spool.tile(
                                [128, 512], F16, tag="paq", bufs=3,
                                name=f"paq{c}_{h}_{t}",
                            )
                            nc.vector.tensor_add(paq, pa_tiles.pop((h, t - 1)), pa)
                            pa_tiles[(h, t)] = paq
                        else:
                            pa_tiles[(h, t)] = pa
                    else:  # diagonal pair
                        d = t - npair
                        offs = (0, 512) if d == 0 else (0, 256)
                        for u in range(2):
                            r = 2 * d + u
                            j = 4 * c + r
                            F = 512 - 128 * r
                            nc.tensor.matmul(
                                s_ps[:, offs[u] : offs[u] + F],
                                kt_sb[:, 128 * j : 128 * (j + 1)],
                                qt_sb[:, h, 512 * (c + 1) - F : 512 * (c + 1)],
                                start=True,
                                stop=True,
                            )
                        W = 896 if d == 0 else 384
                        nc.scalar.activation(
                            p[:, 0:W], s_ps[:, 0:W], ACT_EXP,
                            bias=ebias_sb, scale=SCALE,
                        )
                        for u in range(2):
                            off = offs[u]
                            nc.gpsimd.tensor_mul(
                                p[:, off : off + 128], p[:, off : off + 128], tri_sb
                            )
                    p_tiles[(h, t)] = p

                def emit_ctxden(h, t):
                    if t == 0:
                        ctx_ps = ps.tile(
                            [128, 512], F32, tag="ps", bufs=4, name=f"ctx{c}_{h}"
                        )
                        den_ps = ps.tile(
                            [128, 512], F32, tag="ps", bufs=4, name=f"den{c}_{h}"
                        )
                        state[h] = (ctx_ps, den_ps)
                    ctx_ps, den_ps = state[h]
                    p = p_tiles.pop((h, t))
                    if t < npair:
                        for u in range(2):
                            j = 2 * t + u
                            nc.tensor.matmul(
                                ctx_ps,
                                v_sb[:, j, :],
                                p[:, 512 * u : 512 * (u + 1)],
                                start=(j == 0),
                                stop=False,
                            )
                        if t % 2 == 1:
                            nc.tensor.matmul(
                                den_ps,
                                ones_sb,
                                pa_tiles.pop((h, t)),
                                start=(t == 1),
                                stop=False,
                            )
                    else:
                        d = t - npair
                        offs = (0, 512) if d == 0 else (0, 256)
                        for u in range(2):
                            r = 2 * d + u
                            j = 4 * c + r
                            F = 512 - 128 * r
                            nc.tensor.matmul(
                                ctx_ps[:, 512 - F : 512],
                                v_sb[:, j, :],
                                p[:, offs[u] : offs[u] + F],
                                start=(j == 0),
                                stop=(j == nj - 1),
                            )
                            nc.tensor.matmul(
                                den_ps[:, 512 - F : 512],
                                ones_sb,
                                p[:, offs[u] : offs[u] + F],
                                start=(c == 0 and r == 0),
                                stop=(r == 3),
                            )
                    if t == nunits - 1:
                        # drain den and ctx to SBUF, freeing both banks.
                        # den in fp16 is plenty: den in [1e-3, 250], so
                        # ln(den) picks up <~2e-3 absolute -> <0.2% on rec.
                        den16 = spool.tile(
                            [128, 512], F16, tag="den16", bufs=6, name=f"d16_{c}_{h}"
                        )
                        nc.vector.tensor_copy(den16, den_ps)
                        cxu = spool.tile(
                            [128, 512], F16, tag="cxu", bufs=6, name=f"cxu{c}_{h}"
                        )
                        nc.vector.tensor_copy(cxu, ctx_ps)
                        pend.append((c, h, cxu, den16))
                        del state[h]

                units = [(h, t) for h in range(HQ) for t in range(nunits)]
                n = len(units)
                for i in range(n + lagp):
                    if i < n:
                        emit_scores(*units[i])
                    if i == 0:
                        flush_rope()
                    j = i - lagp
                    if j >= 0:
                        emit_ctxden(*units[j])
                    yield

            def run_gen(gen):
                for _ in gen:
                    pass

            def wo_blocks(c, last=False, vec_drains=False):
                """Incremental emitter for Wo chunk c: 4 sections x 4 blocks;
                each block = 2 single-bank PSUM tiles, 8 matmuls (4 heads x
                2 col-groups), 2 drains. Yields after each block."""
                for s in range(4):
                    tq = 4 * c + s
                    flush_rope()
                    finish_norm()
                    ob = opool.tile([128, 8, 512], F16, tag="ob")
                    for bi in range(4):
                        if bi == 2:
                            finish_norm()
                        half, grp = bi // 2, bi % 2
                        pw = [
                            ps.tile(
                                [128, 512], F32, tag="ps", bufs=4,
                                name=f"wops{tq}_{bi}_{i}",
                            )
                            for i in range(2)
                        ]
                        for h in range(HQ):
                            lhsT = cx_sb[:, h, 128 * tq : 128 * (tq + 1)]
                            for i in range(2):
                                n = 4 * half + 2 * grp + i
                                nc.tensor.matmul(
                                    pw[i],
                                    lhsT,
                                    wo_sb[:, h, 512 * n : 512 * (n + 1)],
                                    start=(h == 0),
                                    stop=(h == HQ - 1),
                                )
                        for i in range(2):
                            n = 4 * half + 2 * grp + i
                            if i == 0 or vec_drains:
                                # in merged phases the scalar engine is busy
                                # with attention exps; drain on vector only
                                nc.vector.tensor_copy(ob[:, n, :], pw[i])
                            else:
                                nc.scalar.activation(ob[:, n, :], pw[i], ACT_COPY)
                        if last and s == 3:
                            # final section: store each block's quarter as
                            # soon as its drains are emitted, so the kernel
                            # tail is one drain + one short store
                            nn = 4 * half + 2 * grp
                            nc.sync.dma_start(
                                outp[
                                    128 * tq : 128 * (tq + 1),
                                    512 * nn : 512 * (nn + 2),
                                ],
                                ob[:, nn : nn + 2, :],
                            )
                        elif grp == 1:
                            # half-row output DMA: starts the store while the
                            # other half still computes
                            nc.sync.dma_start(
                                outp[
                                    128 * tq : 128 * (tq + 1),
                                    2048 * half : 2048 * (half + 1),
                                ],
                                ob[:, 4 * half : 4 * (half + 1), :],
                            )
                        yield

            def merged(wo_c, attn_cs, attn_share):
                """Interleave one Wo chunk (16 blocks) with the chained unit
                streams of one or more attention chunks; attn_share units are
                emitted after each wo block (list of 16 ints)."""
                wg = wo_blocks(wo_c)
                ags = [attn_units(c, lagp=2) for c in attn_cs]

                def steps():
                    for ag in ags:
                        yield from ag

                ag = steps()
                # prime the exp pipeline: two units of scores ahead of the
                # first wo block so the scalar engine starts early
                for _ in range(2):
                    next(ag, None)
                for k in attn_share:
                    next(wg)
                    for _ in range(k):
                        next(ag, None)
                run_gen(ag)
                run_gen(wg)

            # emission order ~ per-engine execution order
            proj_chunk(0)
            proj_chunk(1)
            run_gen(attn_units(0, lagp=3, ps_diag=True))
            proj_chunk(2)
            proj_chunk(3)
            # attn1+attn2 (18+26 pipeline steps) into wo0, attn3 (34) into
            # wo1: their exps run against the Wo phases' idle scalar time
            merged(0, (1, 2), [3] * 12 + [2] * 4)
            merged(1, (3,), [2] * 16)
            run_gen(wo_blocks(2))
            run_gen(wo_blocks(3, last=True))
            while pend:
                finish_norm()

    bass_rust.generate_event_semaphores(nc)
    return nc


_NC = None


def _get_nc():
    global _NC
    if _NC is None:
        _NC = _build_nc()
    return _NC


def _host_inputs(resid, Wq, Wk, Wv, Wo):
    f16 = np.float16
    r2 = np.asarray(resid, dtype=np.float32).reshape(T, D_MODEL)
    rt = np.ascontiguousarray(r2.T).astype(f16)  # [D, T]
    cosT, sinM = _rope_tables()
    f = np.arange(128)[None, :]
    p = np.arange(128)[:, None]
    tri = (p <= f).astype(f16)  # [128, 128]
    Wq = np.asarray(Wq, np.float32)
    Wk = np.asarray(Wk, np.float32)
    Wv = np.asarray(Wv, np.float32)
    Wo = np.asarray(Wo, np.float32)
    in_maps = []
    for i in range(NCORES):
        in_maps.append(
            {
                "rt": rt,
                "wq": np.ascontiguousarray(Wq[:, 512 * i : 512 * (i + 1)]).astype(f16),
                "wk": np.ascontiguousarray(Wk[:, 128 * i : 128 * (i + 1)]).astype(f16),
                "wv": np.ascontiguousarray(Wv[:, 128 * i : 128 * (i + 1)]).astype(f16),
                "wo": np.ascontiguousarray(Wo[512 * i : 512 * (i + 1), :]).astype(f16),
                "cosT": cosT,
                "sinM": sinM,
                "tri": tri,
            }
        )
    return in_maps


def run(resid, Wq, Wk, Wv, Wo, **spmd_kwargs):
    in_maps = _host_inputs(resid, Wq, Wk, Wv, Wo)
    nc = _get_nc()
    res = run_bass_kernel_spmd(nc, in_maps, core_ids=list(range(NCORES)), **spmd_kwargs)
    out = np.zeros((T, D_MODEL), np.float32)
    for rmap in res.results:
        out += rmap["outp"].astype(np.float32)
    return out.reshape(1, T, D_MODEL), res


def kernel(resid, Wq, Wk, Wv, Wo):
    # warm-up execution: activation tables and DMA rings are only guaranteed
    # after one execution has cycled them; the second execution is the
    # validated-correct path.
    run(resid, Wq, Wk, Wv, Wo)
    out, _ = run(resid, Wq, Wk, Wv, Wo)
    return out


# revision 30
# speedup vs baseline: 1.0071x; 1.0071x over previous
"""Llama3 GQA causal attention (B=1, T=2048, D=4096, 32 Q heads / 8 KV heads,
dh=128) on 8 Trainium2 NeuronCores.

Sharding: tensor-parallel over heads. Core i owns KV head i and Q heads
4i..4i+3: Wq/Wk/Wv split column-wise, Wo split row-wise. Each core computes a
partial [T, D] output (rows of Wo for its heads); the host sums the 8 partials.

Device layout notes:
 - resid is transposed on the host to rT [D, T] so every projection matmul has
   its contraction dim (d) on partitions with no on-device transpose.
 - Q/K are produced transposed (Q^T [dh, T]) which is exactly the layout the
   scores matmul wants; scores are computed transposed (S^T [Tk, Tq]) so the
   softmax denominator comes from an all-ones-matrix matmul (which also
   broadcasts it to all 128 partitions) and probabilities can be consumed
   directly by the ctx matmul (ctx^T = V^T @ P^T) with V stationary.
 - everything runs in fp16 (fp32 PSUM accumulation): same PE rate as bf16 but
   8x the mantissa, 4x DVE element-wise rate on SBUF tiles, and fp16 output
   partials halve the output DMA.
 - scores matmuls pack two 512-wide fp32 tiles into one 2-bank [128, 1024]
   PSUM region, so the scalar engine runs ONE exp ACTIVATE per two tiles
   (amortizing the ~230ns per-ACTIVATE overhead).
 - the attention phases are scalar-exp-throughput-bound while the Wo phases
   leave the scalar engine ~70% idle, so attention chunks 2 and 3 are
   EMISSION-INTERLEAVED into Wo chunks 0 and 1: the PE alternates Wo blocks
   and attention units, and the attention exps run against the Wo phase's
   idle scalar time. Attention is pipelined globally across heads (not per
   head), so one head's exp latency is hidden by the next head's scores.
 - the softmax denominator uses an all-ones [128,128] stationary, which both
   sums over keys and broadcasts the result to every partition in the same
   matmul; the reciprocal is computed as exp(-ln(den)) on the scalar engine
   with a manually preloaded natural_log_exp_and_others activation table set
   (covers Exp AND Ln), so the whole kernel needs exactly one
   ACT_TABLE_LOAD. The ln/exp/scale run deferred inside the NEXT phase's
   scalar-idle window.
 - causal structure is exploited at 128-column granularity: the four
   diagonal-region tiles (F = 512-128r) pack into two pair regions; only the
   leading 128 columns of each need the triangular mask (gpsimd).
 - a short warm-up burst of dummy matmuls runs during the ~10us startup DMA
   window so the PE's HAM clock gate is already at 2.4 GHz (not the cold
   1.2 GHz) when the first real matmul issues.
PSUM budget: tag "psq" = 2 bufs x 2 banks (proj q-accs / score pairs / wo is
not using it), tag "ps" = 4 bufs x 1 bank (proj k/v accs, V transposes,
ctx/den, wo blocks). Total exactly 8 banks.
"""

import math
import sys

import numpy as np

sys.path.insert(0, "/opt/trn_rl_repo")

import bass_rust

import concourse.bass as bass
import concourse.mybir as mybir
import concourse.tile as tile
from concourse.bass_utils import run_bass_kernel_spmd
from concourse.hw_specs import get_activation_tables

F16 = mybir.dt.float16
F32 = mybir.dt.float32
ACT_COPY = mybir.ActivationFunctionType.Copy
ACT_EXP = mybir.ActivationFunctionType.Exp
ACT_LN = mybir.ActivationFunctionType.Ln

D_MODEL = 4096
N_HEADS = 32
N_KV = 8
DH = 128
T = 2048
NCORES = 8
HQ = N_HEADS // NCORES  # 4 q heads per core
NT = T // 128  # 16 row tiles
NCH = T // 512  # 4 column chunks
SCALE = 1.0 / math.sqrt(DH)
# softmax bias: p = exp(s*SCALE - EXP_BIAS). Cancels between numerator and
# denominator; keeps exp() inside fp16 range.
EXP_BIAS = -3.5
ROPE = dict(
    rope_theta=500000.0,
    factor=32.0,
    hi_freq_factor=4.0,
    lo_freq_factor=1.0,
    original_context_length=8192,
)


def _rope_tables():
    """cos/sin tables in transposed layout [dh, T]; sin has the rotate-half
    sign folded in (rows 0:64 negated)."""
    idx = np.arange(0, DH, 2, dtype=np.float64) / DH
    freq = (1.0 / (2.0 * math.pi)) * ROPE["rope_theta"] ** (-idx)
    factor, lo, hi = ROPE["factor"], ROPE["lo_freq_factor"], ROPE["hi_freq_factor"]
    L0 = ROPE["original_context_length"]
    freq_low, freq_high = lo / L0, hi / L0
    freq_scaled = np.where(freq < freq_low, freq / factor, freq)
    smooth = np.clip((L0 * freq - lo) / (hi - lo), 0.0, 1.0)
    freq_smooth = (1.0 - smooth) * (freq / factor) + smooth * freq
    is_mid = (freq >= freq_low) & (freq <= freq_high)
    freq = np.where(is_mid, freq_smooth, freq_scaled)
    pos = np.arange(T, dtype=np.float64)
    phase = 2.0 * math.pi * pos[:, None] * freq[None, :]  # [T, 64]
    emb = np.concatenate([phase, phase], axis=-1)  # [T, 128]
    cos = np.cos(emb)
    sin = np.sin(emb)
    cosT = np.ascontiguousarray(cos.T).astype(np.float16)  # [128, T]
    sinT = np.ascontiguousarray(sin.T)
    sinM = sinT.copy()
    sinM[:64] = -sinT[:64]
    return cosT, sinM.astype(np.float16)


def _build_nc():
    nc = bass.Bass()
    rt = nc.dram_tensor("rt", [D_MODEL, T], F16, kind="ExternalInput")
    wq = nc.dram_tensor("wq", [D_MODEL, HQ * DH], F16, kind="ExternalInput")
    wk = nc.dram_tensor("wk", [D_MODEL, DH], F16, kind="ExternalInput")
    wv = nc.dram_tensor("wv", [D_MODEL, DH], F16, kind="ExternalInput")
    wo = nc.dram_tensor("wo", [HQ * DH, D_MODEL], F16, kind="ExternalInput")
    cosT = nc.dram_tensor("cosT", [DH, T], F16, kind="ExternalInput")
    sinM = nc.dram_tensor("sinM", [DH, T], F16, kind="ExternalInput")
    tri = nc.dram_tensor("tri", [128, 128], F16, kind="ExternalInput")
    outp = nc.dram_tensor("outp", [T, D_MODEL], F16, kind="ExternalOutput")

    rt3 = rt.rearrange("(o p) t -> p o t", p=128)  # [128, 32, T]
    wq3 = wq.rearrange("(o p) m -> p o m", p=128)  # [128, 32, 512]
    wk3 = wk.rearrange("(o p) m -> p o m", p=128)  # [128, 32, 128]
    wv3 = wv.rearrange("(o p) m -> p o m", p=128)
    wo3 = wo.rearrange("(o p) n -> p o n", p=128)  # [128, 4, 4096]

    with tile.TileContext(nc) as tc:
        with (
            tc.tile_pool(name="consts", bufs=1) as cpool,
            tc.tile_pool(name="acts", bufs=1) as apool,
            tc.tile_pool(name="rtp", bufs=8) as rpool,
            tc.tile_pool(name="scr", bufs=2) as spool,
            tc.tile_pool(name="pt", bufs=4) as ppool,
            tc.tile_pool(name="ob", bufs=2) as opool,
            tc.tile_pool(name="ps", bufs=1, space="PSUM") as ps,
        ):
            tri_sb = cpool.tile([128, 128], F16)

            def load_consts():
                nc.sync.dma_start(tri_sb, tri[:, :])
            ones_sb = cpool.tile([128, 128], F16)
            nc.gpsimd.memset(ones_sb, 1.0)
            ebias_sb = cpool.tile([128, 1], F32)
            nc.gpsimd.memset(ebias_sb, EXP_BIAS)
            # warm-up operand comes from a tiny DMA (the sync engine is ready
            # ~2us before gpsimd finishes its preamble memsets)
            warm_sb = cpool.tile([128, 128], F16)
            nc.sync.dma_start(warm_sb, tri[:, :])
            # preload the one activation-table set covering every function
            # this kernel uses (Exp, Ln, Copy); the bacc fixpoint pass then
            # inserts no further ACT_TABLE_LOADs.
            combo_id = list(get_activation_tables(nc.m.arch)).index(
                "natural_log_exp_and_others"
            )
            ld = mybir.InstLoadActFuncSet(
                name=nc.get_next_instruction_name(), ins=[], outs=[]
            )
            ld.act_func_set_id = combo_id
            nc.scalar.add_instruction(ld)
            # HAM warm-up: ~3.9us of short dummy matmuls during the startup
            # DMA window. They un-throttle the PE clock gate (1.2 -> 2.4 GHz)
            # before the first real matmul, and end before the first real
            # matmul's inputs have landed.
            warm_ps = ps.tile([128, 512], F32, tag="ps", bufs=4, name="warm")
            for _ in range(36):
                nc.tensor.matmul(
                    warm_ps[:, 0:128], warm_sb, warm_sb, start=True, stop=True
                )

            cos_sb = cpool.tile([DH, T], F16)
            sin_sb = cpool.tile([DH, T], F16)
            wq_sb = cpool.tile([128, 32, HQ * DH], F16)
            wk_sb = cpool.tile([128, 32, DH], F16)
            wv_sb = cpool.tile([128, 32, DH], F16)
            wo_sb = cpool.tile([128, HQ, D_MODEL], F16)

            # activations that persist across phases
            qt_sb = apool.tile([128, HQ, T], F16)  # Q^T per head, rope'd
            kt_sb = apool.tile([128, T], F16)  # K^T, rope'd
            v_sb = apool.tile([128, NT, DH], F16)  # V tiles [tk, j, dh]
            cx_sb = apool.tile([128, HQ, T], F16)  # normalized ctx^T

            # deferred normalization: (c, h, cxu, den16); ln/exp/scale all
            # run at flush time, inside a later phase's scalar-idle window.
            pend = []
            # deferred RoPE tiles, flushed a few per phase.
            rope_pend = []

            def flush_rope(n=1):
                for _ in range(min(n, len(rope_pend))):
                    rope_pend.pop(0)()

            def finish_norm():
                if not pend:
                    return
                c, h, cxu, den16 = pend.pop(0)
                cs = slice(512 * c, 512 * (c + 1))
                # rec = exp(-ln(den)); Ln and Exp share the preloaded table
                # set, so no ACT_TABLE_LOADs are triggered.
                nc.scalar.activation(den16, den16, ACT_LN)
                rec16 = spool.tile(
                    [128, 512], F16, tag="rec16", bufs=2, name=f"r16_{c}_{h}"
                )
                nc.scalar.activation(rec16, den16, ACT_EXP, scale=-1.0)
                # all-SBUF fp16 multiply on the otherwise-idle gpsimd engine
                nc.gpsimd.tensor_mul(cx_sb[:, h, cs], cxu, rec16)

            def proj_chunk(c):
                cs = slice(512 * c, 512 * (c + 1))
                # q accumulators pair-packed into two 2-bank PSUM tiles;
                # k and v accumulators in single-bank tiles
                aq = [
                    ps.tile([128, 1024], F32, tag="psq", bufs=2, name=f"acc{c}_{i}")
                    for i in range(2)
                ]
                ak = ps.tile([128, 512], F32, tag="ps", bufs=4, name=f"acck{c}")
                av = ps.tile([128, 512], F32, tag="ps", bufs=4, name=f"accv{c}")

                def acc(i):
                    if i < 4:
                        return aq[i // 2][:, 512 * (i % 2) : 512 * (i % 2 + 1)]
                    return ak if i == 4 else av

                for o2 in range(16):
                    rtt2 = rpool.tile([128, 2, 512], F16, tag="rt", bufs=7)
                    if c == 0 and o2 == 0:
                        # singles: first matmul's dependency is ~256KB of
                        # DMA, not ~1MB
                        nc.sync.dma_start(rtt2[:, 0, :], rt3[:, 0, cs])
                        nc.sync.dma_start(wq_sb[:, 0, :], wq3[:, 0, :])
                        nc.sync.dma_start(wk_sb[:, 0:1, :], wk3[:, 0:1, :])
                        nc.sync.dma_start(wv_sb[:, 0:1, :], wv3[:, 0:1, :])
                        nc.sync.dma_start(rtt2[:, 1, :], rt3[:, 1, cs])
                        nc.sync.dma_start(wq_sb[:, 1, :], wq3[:, 1, :])
                        nc.sync.dma_start(wk_sb[:, 1:4, :], wk3[:, 1:4, :])
                        nc.sync.dma_start(wv_sb[:, 1:4, :], wv3[:, 1:4, :])
                    else:
                        nc.sync.dma_start(rtt2, rt3[:, 2 * o2 : 2 * o2 + 2, cs])
                        if c == 0 and o2 == 12:
                            # rope tables + mask, issued late in chunk 0 where
                            # the weight streams have finished: off both the
                            # chunk-0 and the chunk-1 rt critical paths
                            load_consts()
                            nc.sync.dma_start(cos_sb, cosT[:, :])
                            nc.sync.dma_start(sin_sb, sinM[:, :])
                        if c == 0:
                            o = 2 * o2
                            nc.sync.dma_start(
                                wq_sb[:, o : o + 2, :], wq3[:, o : o + 2, :]
                            )
                            if o % 4 == 0:
                                nc.sync.dma_start(
                                    wk_sb[:, o : o + 4, :], wk3[:, o : o + 4, :]
                                )
                                nc.sync.dma_start(
                                    wv_sb[:, o : o + 4, :], wv3[:, o : o + 4, :]
                                )
                    for oo in range(2):
                        o = 2 * o2 + oo
                        rtt = rtt2[:, oo, :]
                        st, sp = (o == 0), (o == 31)
                        for h in range(HQ):
                            nc.tensor.matmul(
                                acc(h), wq_sb[:, o, 128 * h : 128 * (h + 1)], rtt,
                                start=st, stop=sp,
                            )
                        nc.tensor.matmul(
                            acc(4), wk_sb[:, o, :], rtt, start=st, stop=sp
                        )
                        nc.tensor.matmul(
                            acc(5), wv_sb[:, o, :], rtt, start=st, stop=sp
                        )
                        if o in (8, 12, 16, 20):
                            finish_norm()  # previous attn chunk's norms
                        if o in (6, 10, 14, 18, 22):
                            flush_rope()  # pending rope tiles
                        if c == 2 and o in (8, 14, 20, 26):
                            hh = (o - 8) // 6
                            nc.sync.dma_start(wo_sb[:, hh, :], wo3[:, hh, :])
                # drain PSUM fast: one cast per accumulator (split
                # scalar/vector), then rope runs on fp16 SBUF tiles
                xq = []
                for idx in range(5):
                    x = spool.tile([128, 512], F16, tag=f"x{idx}")
                    if idx == 0 and c != 3:
                        nc.scalar.activation(x, acc(idx), ACT_COPY)
                    else:
                        # c==3: keep the scalar queue empty -- the merged
                        # phase's attention exps follow immediately
                        nc.vector.tensor_copy(x, acc(idx))
                    xq.append(x)
                vt = spool.tile([128, 512], F16, tag="vt")

                def rope_tile(idx, x=None):
                    def go(x=x):
                        xs = spool.tile([128, 512], F16, tag="xs", name=f"xs{c}_{idx}")
                        nc.vector.tensor_copy(xs[0:64, :], x[64:128, :])
                        nc.vector.tensor_copy(xs[64:128, :], x[0:64, :])
                        t1 = spool.tile([128, 512], F16, tag="t1", name=f"t1{c}_{idx}")
                        nc.vector.tensor_mul(t1, x, cos_sb[:, cs])
                        nc.vector.tensor_mul(xs, xs, sin_sb[:, cs])
                        dst = qt_sb[:, idx, cs] if idx < HQ else kt_sb[:, cs]
                        nc.vector.tensor_add(dst, t1, xs)
                    return go

                for idx in range(5):
                    rope_pend.append(rope_tile(idx, xq[idx]))
                # drain the v accumulator (split scalar/vector, frees its
                # PSUM bank), then V^T -> V via DMA-xbar transposes on the
                # otherwise-idle DMA engines. No PE instruction sits at the
                # phase boundary, so the next phase's matmuls never stall
                # behind the vt drain.
                if c != 3:
                    nc.scalar.activation(vt[:, 0:256], acc(5)[:, 0:256], ACT_COPY)
                else:
                    nc.vector.tensor_copy(vt[:, 0:256], acc(5)[:, 0:256])
                nc.vector.tensor_copy(vt[:, 256:512], acc(5)[:, 256:512])
                for s in range(4):
                    nc.sync.dma_start_transpose(
                        v_sb[:, 4 * c + s, :], vt[:, 128 * s : 128 * (s + 1)]
                    )

            def attn_units(c, lagp, ps_diag=False):
                """Incremental emitter for attention chunk c: a generator
                yielding after each (scores+exp | ctx+den) pipeline step.
                Globally pipelined across heads: unit stream is
                [(h,t) for h in heads for t in units-of-head]."""
                npair = 2 * c  # full pairs per head
                nunits = npair + 2  # + two diagonal pairs
                nj = 4 * (c + 1)
                state = {}  # h -> (ctx_ps, den_ps)
                p_tiles = {}
                pa_tiles = {}

                def emit_scores(h, t):
                    # standalone chunk 0 has no proj competing for "ps", so
                    # its second (narrow) diagonal pair allocates there --
                    # doubling the score-buffer depth of the exp pipeline.
                    if ps_diag and t == npair + 1:
                        s_ps = ps.tile(
                            [128, 512], F32, tag="ps", bufs=4, name=f"s{c}_{h}_{t}"
                        )
                    else:
                        s_ps = ps.tile(
                            [128, 1024], F32, tag="psq", bufs=2, name=f"s{c}_{h}_{t}"
                        )
                    p = ppool.tile([128, 1024], F16, tag="pt", name=f"p{c}_{h}_{t}")
                    if t < npair:  # full pair
                        qs_full = qt_sb[:, h, 512 * c : 512 * (c + 1)]
                        for u in range(2):
                            j = 2 * t + u
                            nc.tensor.matmul(
                                s_ps[:, 512 * u : 512 * (u + 1)],
                                kt_sb[:, 128 * j : 128 * (j + 1)],
                                qs_full,
                                start=True,
                                stop=True,
                            )
                        nc.scalar.activation(
                            p, s_ps, ACT_EXP, bias=ebias_sb, scale=SCALE
                        )
                        pa = spool.tile(
                            [128, 512], F16, tag="pa", bufs=4, name=f"pa{c}_{h}_{t}"
                        )
                        nc.vector.tensor_add(pa, p[:, 0:512], p[:, 512:1024])
                        if t % 2 == 1:
                            paq = spool.tile(
                                [128, 512], F16, tag="paq", bufs=3,
                                name=f"paq{c}_{h}_{t}",
                            )
                            nc.vector.tensor_add(paq, pa_tiles.pop((h, t - 1)), pa)
                            pa_tiles[(h, t)] = paq
                        else:
                            pa_tiles[(h, t)] = pa
                    else:  # diagonal pair
                        d = t - npair
                        offs = (0, 512) if d == 0 else (0, 256)
                        for u in range(2):
                            r = 2 * d + u
                            j = 4 * c + r
                            F = 512 - 128 * r
                            nc.tensor.matmul(
                                s_ps[:, offs[u] : offs[u] + F],
                                kt_sb[:, 128 * j : 128 * (j + 1)],
                                qt_sb[:, h, 512 * (c + 1) - F : 512 * (c + 1)],
                                start=True,
                                stop=True,
                            )
                        W = 896 if d == 0 else 384
                        nc.scalar.activation(
                            p[:, 0:W], s_ps[:, 0:W], ACT_EXP,
                            bias=ebias_sb, scale=SCALE,
                        )
                        for u in range(2):
                            off = offs[u]
                            nc.gpsimd.tensor_mul(
                                p[:, off : off + 128], p[:, off : off + 128], tri_sb
                            )
                    p_tiles[(h, t)] = p

                def emit_ctxden(h, t):
                    if t == 0:
                        ctx_ps = ps.tile(
                            [128, 512], F32, tag="ps", bufs=4, name=f"ctx{c}_{h}"
                        )
                        den_ps = ps.tile(
                            [128, 512], F32, tag="ps", bufs=4, name=f"den{c}_{h}"
                        )
                        state[h] = (ctx_ps, den_ps)
                    ctx_ps, den_ps = state[h]
                    p = p_tiles.pop((h, t))
                    if t < npair:
                        for u in range(2):
                            j = 2 * t + u
                            nc.tensor.matmul(
                                ctx_ps,
                                v_sb[:, j, :],
                                p[:, 512 * u : 512 * (u + 1)],
                                start=(j == 0),
                                stop=False,
                            )
                        if t % 2 == 1:
                            nc.tensor.matmul(
                                den_ps,
                                ones_sb,
                                pa_tiles.pop((h, t)),
                                start=(t == 1),
                                stop=False,
                            )
                    else:
                        d = t - npair
                        offs = (0, 512) if d == 0 else (0, 256)
                        for u in range(2):
                            r = 2 * d + u
                            j = 4 * c + r
                            F = 512 - 128 * r
                            nc.tensor.matmul(
                                ctx_ps[:, 512 - F : 512],
                                v_sb[:, j, :],
                                p[:, offs[u] : offs[u] + F],
                                start=(j == 0),
                                stop=(j == nj - 1),
                            )
                            nc.tensor.matmul(
                                den_ps[:, 512 - F : 512],
                                ones_sb,
                                p[:, offs[u] : offs[u] + F],
                                start=(c == 0 and r == 0),
                                stop=(r == 3),
                            )
                    if t == nunits - 1:
                        # drain den and ctx to SBUF, freeing both banks.
                        # den in fp16 is plenty: den in [1e-3, 250], so
                        # ln(den) picks up <~2e-3 absolute -> <0.2% on rec.
                        den16 = spool.tile(
                            [128, 512], F16, tag="den16", bufs=6, name=f"d16_{c}_{h}"
                        )
                        nc.vector.tensor_copy(den16, den_ps)
                        cxu = spool.tile(
                            [128, 512], F16, tag="cxu", bufs=6, name=f"cxu{c}_{h}"
                        )
                        nc.vector.tensor_copy(cxu, ctx_ps)
                        pend.append((c, h, cxu, den16))
                        del state[h]

                units = [(h, t) for h in range(HQ) for t in range(nunits)]
                n = len(units)
                for i in range(n + lagp):
                    if i < n:
                        emit_scores(*units[i])
                    if i == 0:
                        flush_rope()
                    j = i - lagp
                    if j >= 0:
                        emit_ctxden(*units[j])
                    yield

            def run_gen(gen):
                for _ in gen:
                    pass

            def wo_blocks(c, last=False, vec_drains=False):
                """Incremental emitter for Wo chunk c: 4 sections x 4 blocks;
                each block = 2 single-bank PSUM tiles, 8 matmuls (4 heads x
                2 col-groups), 2 drains. Yields after each block."""
                for s in range(4):
                    tq = 4 * c + s
                    flush_rope()
                    finish_norm()
                    ob = opool.tile([128, 8, 512], F16, tag="ob")
                    for bi in range(4):
                        if bi == 2:
                            finish_norm()
                        half, grp = bi // 2, bi % 2
                        pw = [
                            ps.tile(
                                [128, 512], F32, tag="ps", bufs=4,
                                name=f"wops{tq}_{bi}_{i}",
                            )
                            for i in range(2)
                        ]
                        for h in range(HQ):
                            lhsT = cx_sb[:, h, 128 * tq : 128 * (tq + 1)]
                            for i in range(2):
                                n = 4 * half + 2 * grp + i
                                nc.tensor.matmul(
                                    pw[i],
                                    lhsT,
                                    wo_sb[:, h, 512 * n : 512 * (n + 1)],
                                    start=(h == 0),
                                    stop=(h == HQ - 1),
                                )
                        for i in range(2):
                            n = 4 * half + 2 * grp + i
                            if i == 0 or vec_drains:
                                # in merged phases the scalar engine is busy
                                # with attention exps; drain on vector only
                                nc.vector.tensor_copy(ob[:, n, :], pw[i])
                            else:
                                nc.scalar.activation(ob[:, n, :], pw[i], ACT_COPY)
                        if last and s == 3:
                            # final section: store each block's quarter as
                            # soon as its drains are emitted, so the kernel
                            # tail is one drain + one short store
                            nn = 4 * half + 2 * grp
                            nc.sync.dma_start(
                                outp[
                                    128 * tq : 128 * (tq + 1),
                                    512 * nn : 512 * (nn + 2),
                                ],
                                ob[:, nn : nn + 2, :],
                            )
                        elif grp == 1:
                            # half-row output DMA: starts the store while the
                            # other half still computes
                            nc.sync.dma_start(
                                outp[
                                    128 * tq : 128 * (tq + 1),
                                    2048 * half : 2048 * (half + 1),
                                ],
                                ob[:, 4 * half : 4 * (half + 1), :],
                            )
                        yield

            def merged(wo_c, attn_cs, attn_share):
                """Interleave one Wo chunk (16 blocks) with the chained unit
                streams of one or more attention chunks; attn_share units are
                emitted after each wo block (list of 16 ints)."""
                wg = wo_blocks(wo_c)
                ags = [attn_units(c, lagp=2) for c in attn_cs]

                def steps():
                    for ag in ags:
                        yield from ag

                ag = steps()
                # prime the exp pipeline: two units of scores ahead of the
                # first wo block so the scalar engine starts early
                for _ in range(2):
                    next(ag, None)
                for k in attn_share:
                    next(wg)
                    for _ in range(k):
                        next(ag, None)
                run_gen(ag)
                run_gen(wg)

            # emission order ~ per-engine execution order
            proj_chunk(0)
            proj_chunk(1)
            run_gen(attn_units(0, lagp=3, ps_diag=True))
            proj_chunk(2)
            proj_chunk(3)
            # attn1+attn2 (18+26 pipeline steps) into wo0, attn3 (34) into
            # wo1: their exps run against the Wo phases' idle scalar time
            merged(0, (1, 2), [3] * 12 + [2] * 4)
            merged(1, (3,), [2] * 16)
            run_gen(wo_blocks(2))
            run_gen(wo_blocks(3, last=True))
            while pend:
                finish_norm()

    bass_rust.generate_event_semaphores(nc)
    return nc


_NC = None


def _get_nc():
    global _NC
    if _NC is None:
        _NC = _build_nc()
    return _NC


def _host_inputs(resid, Wq, Wk, Wv, Wo):
    f16 = np.float16
    r2 = np.asarray(resid, dtype=np.float32).reshape(T, D_MODEL)
    rt = np.ascontiguousarray(r2.T).astype(f16)  # [D, T]
    cosT, sinM = _rope_tables()
    f = np.arange(128)[None, :]
    p = np.arange(128)[:, None]
    tri = (p <= f).astype(f16)  # [128, 128]
    Wq = np.asarray(Wq, np.float32)
    Wk = np.asarray(Wk, np.float32)
    Wv = np.asarray(Wv, np.float32)
    Wo = np.asarray(Wo, np.float32)
    in_maps = []
    for i in range(NCORES):
        in_maps.append(
            {
                "rt": rt,
                "wq": np.ascontiguousarray(Wq[:, 512 * i : 512 * (i + 1)]).astype(f16),
                "wk": np.ascontiguousarray(Wk[:, 128 * i : 128 * (i + 1)]).astype(f16),
                "wv": np.ascontiguousarray(Wv[:, 128 * i : 128 * (i + 1)]).astype(f16),
                "wo": np.ascontiguousarray(Wo[512 * i : 512 * (i + 1), :]).astype(f16),
                "cosT": cosT,
                "sinM": sinM,
                "tri": tri,
            }
        )
    return in_maps


def run(resid, Wq, Wk, Wv, Wo, **spmd_kwargs):
    in_maps = _host_inputs(resid, Wq, Wk, Wv, Wo)
    nc = _get_nc()
    res = run_bass_kernel_spmd(nc, in_maps, core_ids=list(range(NCORES)), **spmd_kwargs)
    out = np.zeros((T, D_MODEL), np.float32)
    for rmap in res.results:
        out += rmap["outp"].astype(np.float32)
    return out.reshape(1, T, D_MODEL), res


def kernel(resid, Wq, Wk, Wv, Wo):
    # warm-up execution: activation tables and DMA rings are only guaranteed
    # after one execution has cycled them; the second execution is the
    # validated-correct path.
    run(resid, Wq, Wk, Wv, Wo)
    out, _ = run(resid, Wq, Wk, Wv, Wo)
    return out


# revision 31
# speedup vs baseline: 1.0145x; 1.0074x over previous
"""Llama3 GQA causal attention (B=1, T=2048, D=4096, 32 Q heads / 8 KV heads,
dh=128) on 8 Trainium2 NeuronCores.

Sharding: tensor-parallel over heads. Core i owns KV head i and Q heads
4i..4i+3: Wq/Wk/Wv split column-wise, Wo split row-wise. Each core computes a
partial [T, D] output (rows of Wo for its heads); the host sums the 8 partials.

Device layout notes:
 - resid is transposed on the host to rT [D, T] so every projection matmul has
   its contraction dim (d) on partitions with no on-device transpose.
 - Q/K are produced transposed (Q^T [dh, T]) which is exactly the layout the
   scores matmul wants; scores are computed transposed (S^T [Tk, Tq]) so the
   softmax denominator comes from an all-ones-matrix matmul (which also
   broadcasts it to all 128 partitions) and probabilities can be consumed
   directly by the ctx matmul (ctx^T = V^T @ P^T) with V stationary.
 - everything runs in fp16 (fp32 PSUM accumulation): same PE rate as bf16 but
   8x the mantissa, 4x DVE element-wise rate on SBUF tiles, and fp16 output
   partials halve the output DMA.
 - scores matmuls pack two 512-wide fp32 tiles into one 2-bank [128, 1024]
   PSUM region, so the scalar engine runs ONE exp ACTIVATE per two tiles
   (amortizing the ~230ns per-ACTIVATE overhead).
 - the attention phases are scalar-exp-throughput-bound while the Wo phases
   leave the scalar engine ~70% idle, so attention chunks 2 and 3 are
   EMISSION-INTERLEAVED into Wo chunks 0 and 1: the PE alternates Wo blocks
   and attention units, and the attention exps run against the Wo phase's
   idle scalar time. Attention is pipelined globally across heads (not per
   head), so one head's exp latency is hidden by the next head's scores.
 - the softmax denominator uses an all-ones [128,128] stationary, which both
   sums over keys and broadcasts the result to every partition in the same
   matmul; the reciprocal is computed as exp(-ln(den)) on the scalar engine
   with a manually preloaded natural_log_exp_and_others activation table set
   (covers Exp AND Ln), so the whole kernel needs exactly one
   ACT_TABLE_LOAD. The ln/exp/scale run deferred inside the NEXT phase's
   scalar-idle window.
 - causal structure is exploited at 128-column granularity: the four
   diagonal-region tiles (F = 512-128r) pack into two pair regions; only the
   leading 128 columns of each need the triangular mask (gpsimd).
 - a short warm-up burst of dummy matmuls runs during the ~10us startup DMA
   window so the PE's HAM clock gate is already at 2.4 GHz (not the cold
   1.2 GHz) when the first real matmul issues.
PSUM budget: tag "psq" = 2 bufs x 2 banks (proj q-accs / score pairs / wo is
not using it), tag "ps" = 4 bufs x 1 bank (proj k/v accs, V transposes,
ctx/den, wo blocks). Total exactly 8 banks.
"""

import math
import sys

import numpy as np

sys.path.insert(0, "/opt/trn_rl_repo")

import bass_rust

import concourse.bass as bass
import concourse.mybir as mybir
import concourse.tile as tile
from concourse.bass_utils import run_bass_kernel_spmd
from concourse.hw_specs import get_activation_tables

F16 = mybir.dt.float16
F32 = mybir.dt.float32
ACT_COPY = mybir.ActivationFunctionType.Copy
ACT_EXP = mybir.ActivationFunctionType.Exp
ACT_LN = mybir.ActivationFunctionType.Ln

D_MODEL = 4096
N_HEADS = 32
N_KV = 8
DH = 128
T = 2048
NCORES = 8
HQ = N_HEADS // NCORES  # 4 q heads per core
NT = T // 128  # 16 row tiles
NCH = T // 512  # 4 column chunks
SCALE = 1.0 / math.sqrt(DH)
# softmax bias: p = exp(s*SCALE - EXP_BIAS). Cancels between numerator and
# denominator; keeps exp() inside fp16 range.
EXP_BIAS = -3.5
ROPE = dict(
    rope_theta=500000.0,
    factor=32.0,
    hi_freq_factor=4.0,
    lo_freq_factor=1.0,
    original_context_length=8192,
)


def _rope_tables():
    """cos/sin tables in transposed layout [dh, T]; sin has the rotate-half
    sign folded in (rows 0:64 negated)."""
    idx = np.arange(0, DH, 2, dtype=np.float64) / DH
    freq = (1.0 / (2.0 * math.pi)) * ROPE["rope_theta"] ** (-idx)
    factor, lo, hi = ROPE["factor"], ROPE["lo_freq_factor"], ROPE["hi_freq_factor"]
    L0 = ROPE["original_context_length"]
    freq_low, freq_high = lo / L0, hi / L0
    freq_scaled = np.where(freq < freq_low, freq / factor, freq)
    smooth = np.clip((L0 * freq - lo) / (hi - lo), 0.0, 1.0)
    freq_smooth = (1.0 - smooth) * (freq / factor) + smooth * freq
    is_mid = (freq >= freq_low) & (freq <= freq_high)
    freq = np.where(is_mid, freq_smooth, freq_scaled)
    pos = np.arange(T, dtype=np.float64)
    phase = 2.0 * math.pi * pos[:, None] * freq[None, :]  # [T, 64]
    emb = np.concatenate([phase, phase], axis=-1)  # [T, 128]
    cos = np.cos(emb)
    sin = np.sin(emb)
    cosT = np.ascontiguousarray(cos.T).astype(np.float16)  # [128, T]
    sinT = np.ascontiguousarray(sin.T)
    sinM = sinT.copy()
    sinM[:64] = -sinT[:64]
    return cosT, sinM.astype(np.float16)


def _build_nc():
    nc = bass.Bass()
    rt = nc.dram_tensor("rt", [D_MODEL, T], F16, kind="ExternalInput")
    wq = nc.dram_tensor("wq", [D_MODEL, HQ * DH], F16, kind="ExternalInput")
    wk = nc.dram_tensor("wk", [D_MODEL, DH], F16, kind="ExternalInput")
    wv = nc.dram_tensor("wv", [D_MODEL, DH], F16, kind="ExternalInput")
    wo = nc.dram_tensor("wo", [HQ * DH, D_MODEL], F16, kind="ExternalInput")
    cosT = nc.dram_tensor("cosT", [DH, T], F16, kind="ExternalInput")
    sinM = nc.dram_tensor("sinM", [DH, T], F16, kind="ExternalInput")
    tri = nc.dram_tensor("tri", [128, 128], F16, kind="ExternalInput")
    outp = nc.dram_tensor("outp", [T, D_MODEL], F16, kind="ExternalOutput")

    rt3 = rt.rearrange("(o p) t -> p o t", p=128)  # [128, 32, T]
    wq3 = wq.rearrange("(o p) m -> p o m", p=128)  # [128, 32, 512]
    wk3 = wk.rearrange("(o p) m -> p o m", p=128)  # [128, 32, 128]
    wv3 = wv.rearrange("(o p) m -> p o m", p=128)
    wo3 = wo.rearrange("(o p) n -> p o n", p=128)  # [128, 4, 4096]

    with tile.TileContext(nc) as tc:
        with (
            tc.tile_pool(name="consts", bufs=1) as cpool,
            tc.tile_pool(name="acts", bufs=1) as apool,
            tc.tile_pool(name="rtp", bufs=8) as rpool,
            tc.tile_pool(name="scr", bufs=2) as spool,
            tc.tile_pool(name="pt", bufs=4) as ppool,
            tc.tile_pool(name="ob", bufs=2) as opool,
            tc.tile_pool(name="ps", bufs=1, space="PSUM") as ps,
        ):
            tri_sb = cpool.tile([128, 128], F16)

            def load_consts():
                nc.sync.dma_start(tri_sb, tri[:, :])
            ones_sb = cpool.tile([128, 128], F16)
            nc.gpsimd.memset(ones_sb, 1.0)
            ebias_sb = cpool.tile([128, 1], F32)
            nc.gpsimd.memset(ebias_sb, EXP_BIAS)
            # warm-up operand from a gpsimd memset: measured to unblock the
            # PE ~1.3us earlier than a DMA-fed operand (the DMA queues spin
            # up slower than the gpsimd preamble finishes)
            warm_sb = cpool.tile([128, 128], F16)
            nc.gpsimd.memset(warm_sb, 0.0)
            # preload the one activation-table set covering every function
            # this kernel uses (Exp, Ln, Copy); the bacc fixpoint pass then
            # inserts no further ACT_TABLE_LOADs.
            combo_id = list(get_activation_tables(nc.m.arch)).index(
                "natural_log_exp_and_others"
            )
            ld = mybir.InstLoadActFuncSet(
                name=nc.get_next_instruction_name(), ins=[], outs=[]
            )
            ld.act_func_set_id = combo_id
            nc.scalar.add_instruction(ld)
            # HAM warm-up: ~3.9us of short dummy matmuls during the startup
            # DMA window. They un-throttle the PE clock gate (1.2 -> 2.4 GHz)
            # before the first real matmul, and end before the first real
            # matmul's inputs have landed.
            warm_ps = ps.tile([128, 512], F32, tag="ps", bufs=4, name="warm")
            for _ in range(34):
                nc.tensor.matmul(
                    warm_ps[:, 0:128], warm_sb, warm_sb, start=True, stop=True
                )

            cos_sb = cpool.tile([DH, T], F16)
            sin_sb = cpool.tile([DH, T], F16)
            wq_sb = cpool.tile([128, 32, HQ * DH], F16)
            wk_sb = cpool.tile([128, 32, DH], F16)
            wv_sb = cpool.tile([128, 32, DH], F16)
            wo_sb = cpool.tile([128, HQ, D_MODEL], F16)

            # activations that persist across phases
            qt_sb = apool.tile([128, HQ, T], F16)  # Q^T per head, rope'd
            kt_sb = apool.tile([128, T], F16)  # K^T, rope'd
            v_sb = apool.tile([128, NT, DH], F16)  # V tiles [tk, j, dh]
            cx_sb = apool.tile([128, HQ, T], F16)  # normalized ctx^T

            # deferred normalization: (c, h, cxu, den16); ln/exp/scale all
            # run at flush time, inside a later phase's scalar-idle window.
            pend = []
            # deferred RoPE tiles, flushed a few per phase.
            rope_pend = []

            def flush_rope(n=1):
                for _ in range(min(n, len(rope_pend))):
                    rope_pend.pop(0)()

            def finish_norm():
                if not pend:
                    return
                c, h, cxu, den16 = pend.pop(0)
                cs = slice(512 * c, 512 * (c + 1))
                # rec = exp(-ln(den)); Ln and Exp share the preloaded table
                # set, so no ACT_TABLE_LOADs are triggered.
                nc.scalar.activation(den16, den16, ACT_LN)
                rec16 = spool.tile(
                    [128, 512], F16, tag="rec16", bufs=2, name=f"r16_{c}_{h}"
                )
                nc.scalar.activation(rec16, den16, ACT_EXP, scale=-1.0)
                # all-SBUF fp16 multiply on the otherwise-idle gpsimd engine
                nc.gpsimd.tensor_mul(cx_sb[:, h, cs], cxu, rec16)

            def proj_chunk(c):
                cs = slice(512 * c, 512 * (c + 1))
                # q accumulators pair-packed into two 2-bank PSUM tiles;
                # k and v accumulators in single-bank tiles
                aq = [
                    ps.tile([128, 1024], F32, tag="psq", bufs=2, name=f"acc{c}_{i}")
                    for i in range(2)
                ]
                ak = ps.tile([128, 512], F32, tag="ps", bufs=4, name=f"acck{c}")
                av = ps.tile([128, 512], F32, tag="ps", bufs=4, name=f"accv{c}")

                def acc(i):
                    if i < 4:
                        return aq[i // 2][:, 512 * (i % 2) : 512 * (i % 2 + 1)]
                    return ak if i == 4 else av

                for o2 in range(16):
                    rtt2 = rpool.tile([128, 2, 512], F16, tag="rt", bufs=7)
                    if c == 0 and o2 == 0:
                        # singles: first matmul's dependency is ~256KB of
                        # DMA, not ~1MB
                        nc.sync.dma_start(rtt2[:, 0, :], rt3[:, 0, cs])
                        nc.sync.dma_start(wq_sb[:, 0, :], wq3[:, 0, :])
                        nc.sync.dma_start(wk_sb[:, 0:1, :], wk3[:, 0:1, :])
                        nc.sync.dma_start(wv_sb[:, 0:1, :], wv3[:, 0:1, :])
                        nc.sync.dma_start(rtt2[:, 1, :], rt3[:, 1, cs])
                        nc.sync.dma_start(wq_sb[:, 1, :], wq3[:, 1, :])
                        nc.sync.dma_start(wk_sb[:, 1:4, :], wk3[:, 1:4, :])
                        nc.sync.dma_start(wv_sb[:, 1:4, :], wv3[:, 1:4, :])
                    else:
                        nc.sync.dma_start(rtt2, rt3[:, 2 * o2 : 2 * o2 + 2, cs])
                        if c == 0 and o2 == 12:
                            # rope tables + mask, issued late in chunk 0 where
                            # the weight streams have finished: off both the
                            # chunk-0 and the chunk-1 rt critical paths
                            load_consts()
                            nc.sync.dma_start(cos_sb, cosT[:, :])
                            nc.sync.dma_start(sin_sb, sinM[:, :])
                        if c == 0:
                            o = 2 * o2
                            nc.sync.dma_start(
                                wq_sb[:, o : o + 2, :], wq3[:, o : o + 2, :]
                            )
                            if o % 4 == 0:
                                nc.sync.dma_start(
                                    wk_sb[:, o : o + 4, :], wk3[:, o : o + 4, :]
                                )
                                nc.sync.dma_start(
                                    wv_sb[:, o : o + 4, :], wv3[:, o : o + 4, :]
                                )
                    for oo in range(2):
                        o = 2 * o2 + oo
                        rtt = rtt2[:, oo, :]
                        st, sp = (o == 0), (o == 31)
                        for h in range(HQ):
                            nc.tensor.matmul(
                                acc(h), wq_sb[:, o, 128 * h : 128 * (h + 1)], rtt,
                                start=st, stop=sp,
                            )
                        nc.tensor.matmul(
                            acc(4), wk_sb[:, o, :], rtt, start=st, stop=sp
                        )
                        nc.tensor.matmul(
                            acc(5), wv_sb[:, o, :], rtt, start=st, stop=sp
                        )
                        if o in (8, 12, 16, 20):
                            finish_norm()  # previous attn chunk's norms
                        if o in (6, 10, 14, 18, 22):
                            flush_rope()  # pending rope tiles
                        if c == 2 and o in (8, 14, 20, 26):
                            hh = (o - 8) // 6
                            nc.sync.dma_start(wo_sb[:, hh, :], wo3[:, hh, :])
                # drain PSUM fast: one cast per accumulator (split
                # scalar/vector), then rope runs on fp16 SBUF tiles
                xq = []
                for idx in range(5):
                    x = spool.tile([128, 512], F16, tag=f"x{idx}")
                    if idx == 0 and c != 3:
                        nc.scalar.activation(x, acc(idx), ACT_COPY)
                    else:
                        # c==3: keep the scalar queue empty -- the merged
                        # phase's attention exps follow immediately
                        nc.vector.tensor_copy(x, acc(idx))
                    xq.append(x)
                vt = spool.tile([128, 512], F16, tag="vt")

                def rope_tile(idx, x=None):
                    def go(x=x):
                        xs = spool.tile([128, 512], F16, tag="xs", name=f"xs{c}_{idx}")
                        nc.vector.tensor_copy(xs[0:64, :], x[64:128, :])
                        nc.vector.tensor_copy(xs[64:128, :], x[0:64, :])
                        t1 = spool.tile([128, 512], F16, tag="t1", name=f"t1{c}_{idx}")
                        nc.vector.tensor_mul(t1, x, cos_sb[:, cs])
                        nc.vector.tensor_mul(xs, xs, sin_sb[:, cs])
                        dst = qt_sb[:, idx, cs] if idx < HQ else kt_sb[:, cs]
                        nc.vector.tensor_add(dst, t1, xs)
                    return go

                for idx in range(5):
                    rope_pend.append(rope_tile(idx, xq[idx]))
                # drain the v accumulator (split scalar/vector, frees its
                # PSUM bank), then V^T -> V via DMA-xbar transposes on the
                # otherwise-idle DMA engines. No PE instruction sits at the
                # phase boundary, so the next phase's matmuls never stall
                # behind the vt drain.
                if c != 3:
                    nc.scalar.activation(vt[:, 0:256], acc(5)[:, 0:256], ACT_COPY)
                else:
                    nc.vector.tensor_copy(vt[:, 0:256], acc(5)[:, 0:256])
                nc.vector.tensor_copy(vt[:, 256:512], acc(5)[:, 256:512])
                for s in range(4):
                    nc.sync.dma_start_transpose(
                        v_sb[:, 4 * c + s, :], vt[:, 128 * s : 128 * (s + 1)]
                    )

            def attn_units(c, lagp, ps_diag=False):
                """Incremental emitter for attention chunk c: a generator
                yielding after each (scores+exp | ctx+den) pipeline step.
                Globally pipelined across heads: unit stream is
                [(h,t) for h in heads for t in units-of-head]."""
                npair = 2 * c  # full pairs per head
                nunits = npair + 2  # + two diagonal pairs
                nj = 4 * (c + 1)
                state = {}  # h -> (ctx_ps, den_ps)
                p_tiles = {}
                pa_tiles = {}

                def emit_scores(h, t):
                    # standalone chunk 0 has no proj competing for "ps", so
                    # its second (narrow) diagonal pair allocates there --
                    # doubling the score-buffer depth of the exp pipeline.
                    if ps_diag and t == npair + 1:
                        s_ps = ps.tile(
                            [128, 512], F32, tag="ps", bufs=4, name=f"s{c}_{h}_{t}"
                        )
                    else:
                        s_ps = ps.tile(
                            [128, 1024], F32, tag="psq", bufs=2, name=f"s{c}_{h}_{t}"
                        )
                    p = ppool.tile([128, 1024], F16, tag="pt", name=f"p{c}_{h}_{t}")
                    if t < npair:  # full pair
                        qs_full = qt_sb[:, h, 512 * c : 512 * (c + 1)]
                        for u in range(2):
                            j = 2 * t + u
                            nc.tensor.matmul(
                                s_ps[:, 512 * u : 512 * (u + 1)],
                                kt_sb[:, 128 * j : 128 * (j + 1)],
                                qs_full,
                                start=True,
                                stop=True,
                            )
                        nc.scalar.activation(
                            p, s_ps, ACT_EXP, bias=ebias_sb, scale=SCALE
                        )
                        pa = spool.tile(
                            [128, 512], F16, tag="pa", bufs=4, name=f"pa{c}_{h}_{t}"
                        )
                        nc.vector.tensor_add(pa, p[:, 0:512], p[:, 512:1024])
                        if t % 2 == 1:
                            paq = spool.tile(
                                [128, 512], F16, tag="paq", bufs=3,
                                name=f"paq{c}_{h}_{t}",
                            )
                            nc.vector.tensor_add(paq, pa_tiles.pop((h, t - 1)), pa)
                            pa_tiles[(h, t)] = paq
                        else:
                            pa_tiles[(h, t)] = pa
                    else:  # diagonal pair
                        d = t - npair
                        offs = (0, 512) if d == 0 else (0, 256)
                        for u in range(2):
                            r = 2 * d + u
                            j = 4 * c + r
                            F = 512 - 128 * r
                            nc.tensor.matmul(
                                s_ps[:, offs[u] : offs[u] + F],
                                kt_sb[:, 128 * j : 128 * (j + 1)],
                                qt_sb[:, h, 512 * (c + 1) - F : 512 * (c + 1)],
                                start=True,
                                stop=True,
                            )
                        W = 896 if d == 0 else 384
                        nc.scalar.activation(
                            p[:, 0:W], s_ps[:, 0:W], ACT_EXP,
                            bias=ebias_sb, scale=SCALE,
                        )
                        for u in range(2):
                            off = offs[u]
                            nc.gpsimd.tensor_mul(
                                p[:, off : off + 128], p[:, off : off + 128], tri_sb
                            )
                    p_tiles[(h, t)] = p

                def emit_ctxden(h, t):
                    if t == 0:
                        ctx_ps = ps.tile(
                            [128, 512], F32, tag="ps", bufs=4, name=f"ctx{c}_{h}"
                        )
                        den_ps = ps.tile(
                            [128, 512], F32, tag="ps", bufs=4, name=f"den{c}_{h}"
                        )
                        state[h] = (ctx_ps, den_ps)
                    ctx_ps, den_ps = state[h]
                    p = p_tiles.pop((h, t))
                    if t < npair:
                        for u in range(2):
                            j = 2 * t + u
                            nc.tensor.matmul(
                                ctx_ps,
                                v_sb[:, j, :],
                                p[:, 512 * u : 512 * (u + 1)],
                                start=(j == 0),
                                stop=False,
                            )
                        if t % 2 == 1:
                            nc.tensor.matmul(
                                den_ps,
                                ones_sb,
                                pa_tiles.pop((h, t)),
                                start=(t == 1),
                                stop=False,
                            )
                    else:
                        d = t - npair
                        offs = (0, 512) if d == 0 else (0, 256)
                        for u in range(2):
                            r = 2 * d + u
                            j = 4 * c + r
                            F = 512 - 128 * r
                            nc.tensor.matmul(
                                ctx_ps[:, 512 - F : 512],
                                v_sb[:, j, :],
                                p[:, offs[u] : offs[u] + F],
                                start=(j == 0),
                                stop=(j == nj - 1),
                            )
                            nc.tensor.matmul(
                                den_ps[:, 512 - F : 512],
                                ones_sb,
                                p[:, offs[u] : offs[u] + F],
                                start=(c == 0 and r == 0),
                                stop=(r == 3),
                            )
                    if t == nunits - 1:
                        # drain den and ctx to SBUF, freeing both banks.
                        # den in fp16 is plenty: den in [1e-3, 250], so
                        # ln(den) picks up <~2e-3 absolute -> <0.2% on rec.
                        den16 = spool.tile(
                            [128, 512], F16, tag="den16", bufs=6, name=f"d16_{c}_{h}"
                        )
                        nc.vector.tensor_copy(den16, den_ps)
                        cxu = spool.tile(
                            [128, 512], F16, tag="cxu", bufs=6, name=f"cxu{c}_{h}"
                        )
                        nc.vector.tensor_copy(cxu, ctx_ps)
                        pend.append((c, h, cxu, den16))
                        del state[h]

                units = [(h, t) for h in range(HQ) for t in range(nunits)]
                n = len(units)
                for i in range(n + lagp):
                    if i < n:
                        emit_scores(*units[i])
                    if i == 0:
                        flush_rope()
                    j = i - lagp
                    if j >= 0:
                        emit_ctxden(*units[j])
                    yield

            def run_gen(gen):
                for _ in gen:
                    pass

            def wo_blocks(c, last=False, vec_drains=False):
                """Incremental emitter for Wo chunk c: 4 sections x 4 blocks;
                each block = 2 single-bank PSUM tiles, 8 matmuls (4 heads x
                2 col-groups), 2 drains. Yields after each block."""
                for s in range(4):
                    tq = 4 * c + s
                    flush_rope()
                    finish_norm()
                    ob = opool.tile([128, 8, 512], F16, tag="ob")
                    for bi in range(4):
                        if bi == 2:
                            finish_norm()
                        half, grp = bi // 2, bi % 2
                        pw = [
                            ps.tile(
                                [128, 512], F32, tag="ps", bufs=4,
                                name=f"wops{tq}_{bi}_{i}",
                            )
                            for i in range(2)
                        ]
                        for h in range(HQ):
                            lhsT = cx_sb[:, h, 128 * tq : 128 * (tq + 1)]
                            for i in range(2):
                                n = 4 * half + 2 * grp + i
                                nc.tensor.matmul(
                                    pw[i],
                                    lhsT,
                                    wo_sb[:, h, 512 * n : 512 * (n + 1)],
                                    start=(h == 0),
                                    stop=(h == HQ - 1),
                                )
                        for i in range(2):
                            n = 4 * half + 2 * grp + i
                            if i == 0 or vec_drains:
                                # in merged phases the scalar engine is busy
                                # with attention exps; drain on vector only
                                nc.vector.tensor_copy(ob[:, n, :], pw[i])
                            else:
                                nc.scalar.activation(ob[:, n, :], pw[i], ACT_COPY)
                        if last and s == 3:
                            # final section: store each block's quarter as
                            # soon as its drains are emitted, so the kernel
                            # tail is one drain + one short store
                            nn = 4 * half + 2 * grp
                            nc.sync.dma_start(
                                outp[
                                    128 * tq : 128 * (tq + 1),
                                    512 * nn : 512 * (nn + 2),
                                ],
                                ob[:, nn : nn + 2, :],
                            )
                        elif grp == 1:
                            # half-row output DMA: starts the store while the
                            # other half still computes
                            nc.sync.dma_start(
                                outp[
                                    128 * tq : 128 * (tq + 1),
                                    2048 * half : 2048 * (half + 1),
                                ],
                                ob[:, 4 * half : 4 * (half + 1), :],
                            )
                        yield

            def merged(wo_c, attn_cs, attn_share):
                """Interleave one Wo chunk (16 blocks) with the chained unit
                streams of one or more attention chunks; attn_share units are
                emitted after each wo block (list of 16 ints)."""
                wg = wo_blocks(wo_c)
                ags = [attn_units(c, lagp=2) for c in attn_cs]

                def steps():
                    for ag in ags:
                        yield from ag

                ag = steps()
                # prime the exp pipeline: two units of scores ahead of the
                # first wo block so the scalar engine starts early
                for _ in range(2):
                    next(ag, None)
                for k in attn_share:
                    next(wg)
                    for _ in range(k):
                        next(ag, None)
                run_gen(ag)
                run_gen(wg)

            # emission order ~ per-engine execution order
            proj_chunk(0)
            proj_chunk(1)
            run_gen(attn_units(0, lagp=3, ps_diag=True))
            proj_chunk(2)
            proj_chunk(3)
            # attn1+attn2 (18+26 pipeline steps) into wo0, attn3 (34) into
            # wo1: their exps run against the Wo phases' idle scalar time
            merged(0, (1, 2), [3] * 12 + [2] * 4)
            merged(1, (3,), [2] * 16)
            run_gen(wo_blocks(2))
            run_gen(wo_blocks(3, last=True))
            while pend:
                finish_norm()

    bass_rust.generate_event_semaphores(nc)
    return nc


_NC = None


def _get_nc():
    global _NC
    if _NC is None:
        _NC = _build_nc()
    return _NC


def _host_inputs(resid, Wq, Wk, Wv, Wo):
    f16 = np.float16
    r2 = np.asarray(resid, dtype=np.float32).reshape(T, D_MODEL)
    rt = np.ascontiguousarray(r2.T).astype(f16)  # [D, T]
    cosT, sinM = _rope_tables()
    f = np.arange(128)[None, :]
    p = np.arange(128)[:, None]
    tri = (p <= f).astype(f16)  # [128, 128]
    Wq = np.asarray(Wq, np.float32)
    Wk = np.asarray(Wk, np.float32)
    Wv = np.asarray(Wv, np.float32)
    Wo = np.asarray(Wo, np.float32)
    in_maps = []
    for i in range(NCORES):
        in_maps.append(
            {
                "rt": rt,
                "wq": np.ascontiguousarray(Wq[:, 512 * i : 512 * (i + 1)]).astype(f16),
                "wk": np.ascontiguousarray(Wk[:, 128 * i : 128 * (i + 1)]).astype(f16),
                "wv": np.ascontiguousarray(Wv[:, 128 * i : 128 * (i + 1)]).astype(f16),
                "wo": np.ascontiguousarray(Wo[512 * i : 512 * (i + 1), :]).astype(f16),
                "cosT": cosT,
                "sinM": sinM,
                "tri": tri,
            }
        )
    return in_maps


def run(resid, Wq, Wk, Wv, Wo, **spmd_kwargs):
    in_maps = _host_inputs(resid, Wq, Wk, Wv, Wo)
    nc = _get_nc()
    res = run_bass_kernel_spmd(nc, in_maps, core_ids=list(range(NCORES)), **spmd_kwargs)
    out = np.zeros((T, D_MODEL), np.float32)
    for rmap in res.results:
        out += rmap["outp"].astype(np.float32)
    return out.reshape(1, T, D_MODEL), res


def kernel(resid, Wq, Wk, Wv, Wo):
    # warm-up execution: activation tables and DMA rings are only guaranteed
    # after one execution has cycled them; the second execution is the
    # validated-correct path.
    run(resid, Wq, Wk, Wv, Wo)
    out, _ = run(resid, Wq, Wk, Wv, Wo)
    return out


# revision 32
# speedup vs baseline: 1.0302x; 1.0155x over previous
"""Llama3 GQA causal attention (B=1, T=2048, D=4096, 32 Q heads / 8 KV heads,
dh=128) on 8 Trainium2 NeuronCores.

Sharding: tensor-parallel over heads. Core i owns KV head i and Q heads
4i..4i+3: Wq/Wk/Wv split column-wise, Wo split row-wise. Each core computes a
partial [T, D] output (rows of Wo for its heads); the host sums the 8 partials.

Device layout notes:
 - resid is transposed on the host to rT [D, T] so every projection matmul has
   its contraction dim (d) on partitions with no on-device transpose.
 - Q/K are produced transposed (Q^T [dh, T]) which is exactly the layout the
   scores matmul wants; scores are computed transposed (S^T [Tk, Tq]) so the
   softmax denominator comes from an all-ones-matrix matmul (which also
   broadcasts it to all 128 partitions) and probabilities can be consumed
   directly by the ctx matmul (ctx^T = V^T @ P^T) with V stationary.
 - everything runs in fp16 (fp32 PSUM accumulation): same PE rate as bf16 but
   8x the mantissa, 4x DVE element-wise rate on SBUF tiles, and fp16 output
   partials halve the output DMA.
 - scores matmuls pack two 512-wide fp32 tiles into one 2-bank [128, 1024]
   PSUM region, so the scalar engine runs ONE exp ACTIVATE per two tiles
   (amortizing the ~230ns per-ACTIVATE overhead).
 - the attention phases are scalar-exp-throughput-bound while the Wo phases
   leave the scalar engine ~70% idle, so attention chunks 2 and 3 are
   EMISSION-INTERLEAVED into Wo chunks 0 and 1: the PE alternates Wo blocks
   and attention units, and the attention exps run against the Wo phase's
   idle scalar time. Attention is pipelined globally across heads (not per
   head), so one head's exp latency is hidden by the next head's scores.
 - the softmax denominator uses an all-ones [128,128] stationary, which both
   sums over keys and broadcasts the result to every partition in the same
   matmul; the reciprocal is computed as exp(-ln(den)) on the scalar engine
   with a manually preloaded natural_log_exp_and_others activation table set
   (covers Exp AND Ln), so the whole kernel needs exactly one
   ACT_TABLE_LOAD. The ln/exp/scale run deferred inside the NEXT phase's
   scalar-idle window.
 - causal structure is exploited at 128-column granularity: the four
   diagonal-region tiles (F = 512-128r) pack into two pair regions; only the
   leading 128 columns of each need the triangular mask (gpsimd).
 - a short warm-up burst of dummy matmuls runs during the ~10us startup DMA
   window so the PE's HAM clock gate is already at 2.4 GHz (not the cold
   1.2 GHz) when the first real matmul issues.
PSUM budget: tag "psq" = 2 bufs x 2 banks (proj q-accs / score pairs / wo is
not using it), tag "ps" = 4 bufs x 1 bank (proj k/v accs, V transposes,
ctx/den, wo blocks). Total exactly 8 banks.
"""

import math
import sys

import numpy as np

sys.path.insert(0, "/opt/trn_rl_repo")

import bass_rust

import concourse.bass as bass
import concourse.mybir as mybir
import concourse.tile as tile
from concourse.bass_utils import run_bass_kernel_spmd
from concourse.hw_specs import get_activation_tables

F16 = mybir.dt.float16
F32 = mybir.dt.float32
ACT_COPY = mybir.ActivationFunctionType.Copy
ACT_EXP = mybir.ActivationFunctionType.Exp
ACT_LN = mybir.ActivationFunctionType.Ln

D_MODEL = 4096
N_HEADS = 32
N_KV = 8
DH = 128
T = 2048
NCORES = 8
HQ = N_HEADS // NCORES  # 4 q heads per core
NT = T // 128  # 16 row tiles
NCH = T // 512  # 4 column chunks
SCALE = 1.0 / math.sqrt(DH)
# softmax bias: p = exp(s*SCALE - EXP_BIAS). Cancels between numerator and
# denominator; keeps exp() inside fp16 range.
EXP_BIAS = -3.5
ROPE = dict(
    rope_theta=500000.0,
    factor=32.0,
    hi_freq_factor=4.0,
    lo_freq_factor=1.0,
    original_context_length=8192,
)


def _rope_tables():
    """cos/sin tables in transposed layout [dh, T]; sin has the rotate-half
    sign folded in (rows 0:64 negated)."""
    idx = np.arange(0, DH, 2, dtype=np.float64) / DH
    freq = (1.0 / (2.0 * math.pi)) * ROPE["rope_theta"] ** (-idx)
    factor, lo, hi = ROPE["factor"], ROPE["lo_freq_factor"], ROPE["hi_freq_factor"]
    L0 = ROPE["original_context_length"]
    freq_low, freq_high = lo / L0, hi / L0
    freq_scaled = np.where(freq < freq_low, freq / factor, freq)
    smooth = np.clip((L0 * freq - lo) / (hi - lo), 0.0, 1.0)
    freq_smooth = (1.0 - smooth) * (freq / factor) + smooth * freq
    is_mid = (freq >= freq_low) & (freq <= freq_high)
    freq = np.where(is_mid, freq_smooth, freq_scaled)
    pos = np.arange(T, dtype=np.float64)
    phase = 2.0 * math.pi * pos[:, None] * freq[None, :]  # [T, 64]
    emb = np.concatenate([phase, phase], axis=-1)  # [T, 128]
    cos = np.cos(emb)
    sin = np.sin(emb)
    cosT = np.ascontiguousarray(cos.T).astype(np.float16)  # [128, T]
    sinT = np.ascontiguousarray(sin.T)
    sinM = sinT.copy()
    sinM[:64] = -sinT[:64]
    return cosT, sinM.astype(np.float16)


def _build_nc():
    nc = bass.Bass()
    rt = nc.dram_tensor("rt", [D_MODEL, T], F16, kind="ExternalInput")
    wq = nc.dram_tensor("wq", [D_MODEL, HQ * DH], F16, kind="ExternalInput")
    wk = nc.dram_tensor("wk", [D_MODEL, DH], F16, kind="ExternalInput")
    wv = nc.dram_tensor("wv", [D_MODEL, DH], F16, kind="ExternalInput")
    wo = nc.dram_tensor("wo", [HQ * DH, D_MODEL], F16, kind="ExternalInput")
    cosT = nc.dram_tensor("cosT", [DH, T], F16, kind="ExternalInput")
    sinM = nc.dram_tensor("sinM", [DH, T], F16, kind="ExternalInput")
    tri = nc.dram_tensor("tri", [128, 128], F16, kind="ExternalInput")
    outp = nc.dram_tensor("outp", [T, D_MODEL], F16, kind="ExternalOutput")

    rt3 = rt.rearrange("(o p) t -> p o t", p=128)  # [128, 32, T]
    wq3 = wq.rearrange("(o p) m -> p o m", p=128)  # [128, 32, 512]
    wk3 = wk.rearrange("(o p) m -> p o m", p=128)  # [128, 32, 128]
    wv3 = wv.rearrange("(o p) m -> p o m", p=128)
    wo3 = wo.rearrange("(o p) n -> p o n", p=128)  # [128, 4, 4096]

    with tile.TileContext(nc) as tc:
        with (
            tc.tile_pool(name="consts", bufs=1) as cpool,
            tc.tile_pool(name="acts", bufs=1) as apool,
            tc.tile_pool(name="rtp", bufs=8) as rpool,
            tc.tile_pool(name="scr", bufs=2) as spool,
            tc.tile_pool(name="pt", bufs=4) as ppool,
            tc.tile_pool(name="ob", bufs=2) as opool,
            tc.tile_pool(name="ps", bufs=1, space="PSUM") as ps,
        ):
            tri_sb = cpool.tile([128, 128], F16)

            def load_consts():
                nc.sync.dma_start(tri_sb, tri[:, :])
            ones_sb = cpool.tile([128, 128], F16)
            nc.gpsimd.memset(ones_sb, 1.0)
            ebias_sb = cpool.tile([128, 1], F32)
            nc.gpsimd.memset(ebias_sb, EXP_BIAS)
            # warm-up operand from a gpsimd memset: measured to unblock the
            # PE ~1.3us earlier than a DMA-fed operand (the DMA queues spin
            # up slower than the gpsimd preamble finishes)
            warm_sb = cpool.tile([128, 128], F16)
            nc.gpsimd.memset(warm_sb, 0.0)
            # preload the one activation-table set covering every function
            # this kernel uses (Exp, Ln, Copy); the bacc fixpoint pass then
            # inserts no further ACT_TABLE_LOADs.
            combo_id = list(get_activation_tables(nc.m.arch)).index(
                "natural_log_exp_and_others"
            )
            ld = mybir.InstLoadActFuncSet(
                name=nc.get_next_instruction_name(), ins=[], outs=[]
            )
            ld.act_func_set_id = combo_id
            nc.scalar.add_instruction(ld)
            # HAM warm-up: ~3.9us of short dummy matmuls during the startup
            # DMA window. They un-throttle the PE clock gate (1.2 -> 2.4 GHz)
            # before the first real matmul, and end before the first real
            # matmul's inputs have landed.
            warm_ps = ps.tile([128, 512], F32, tag="ps", bufs=4, name="warm")
            for _ in range(34):
                nc.tensor.matmul(
                    warm_ps[:, 0:128], warm_sb, warm_sb, start=True, stop=True
                )

            cos_sb = cpool.tile([DH, T], F16)
            sin_sb = cpool.tile([DH, T], F16)
            wq_sb = cpool.tile([128, 32, HQ * DH], F16)
            wk_sb = cpool.tile([128, 32, DH], F16)
            wv_sb = cpool.tile([128, 32, DH], F16)
            wo_sb = cpool.tile([128, HQ, D_MODEL], F16)

            # activations that persist across phases
            qt_sb = apool.tile([128, HQ, T], F16)  # Q^T per head, rope'd
            kt_sb = apool.tile([128, T], F16)  # K^T, rope'd
            v_sb = apool.tile([128, NT, DH], F16)  # V tiles [tk, j, dh]
            cx_sb = apool.tile([128, HQ, T], F16)  # normalized ctx^T

            # deferred normalization: (c, h, cxu, den16); ln/exp/scale all
            # run at flush time, inside a later phase's scalar-idle window.
            pend = []
            # deferred RoPE tiles, flushed a few per phase.
            rope_pend = []

            def flush_rope(n=1):
                for _ in range(min(n, len(rope_pend))):
                    rope_pend.pop(0)()

            def finish_norm():
                if not pend:
                    return
                c, h, cxu, den16 = pend.pop(0)
                cs = slice(512 * c, 512 * (c + 1))
                # rec = exp(-ln(den)); Ln and Exp share the preloaded table
                # set, so no ACT_TABLE_LOADs are triggered.
                nc.scalar.activation(den16, den16, ACT_LN)
                rec16 = spool.tile(
                    [128, 512], F16, tag="rec16", bufs=2, name=f"r16_{c}_{h}"
                )
                nc.scalar.activation(rec16, den16, ACT_EXP, scale=-1.0)
                # all-SBUF fp16 multiply on the otherwise-idle gpsimd engine
                nc.gpsimd.tensor_mul(cx_sb[:, h, cs], cxu, rec16)

            def proj_chunk(c):
                cs = slice(512 * c, 512 * (c + 1))
                # q accumulators pair-packed into two 2-bank PSUM tiles;
                # k and v accumulators in single-bank tiles
                aq = [
                    ps.tile([128, 1024], F32, tag="psq", bufs=2, name=f"acc{c}_{i}")
                    for i in range(2)
                ]
                ak = ps.tile([128, 512], F32, tag="ps", bufs=4, name=f"acck{c}")
                av = ps.tile([128, 512], F32, tag="ps", bufs=4, name=f"accv{c}")

                # attention chunk 0 rides inside proj2: its 12 score tiles
                # (3 single-bank pieces per head) use the two "ps" banks the
                # k/v accumulators leave free, its exps use proj2's idle
                # scalar time, and its ctx/den matmuls run as a short tail
                # after the accumulator drains free their banks.
                a0_pieces = []
                a0_p = {}
                if c == 2:
                    def a0_score_piece(h, i):
                        def go():
                            S = ps.tile(
                                [128, 512], F32, tag="ps", bufs=4,
                                name=f"a0s{h}_{i}",
                            )
                            if i == 0:
                                a0_p[h] = ppool.tile(
                                    [128, 1280], F16, tag="p0", bufs=4,
                                    name=f"a0p{h}",
                                )
                            pt0 = a0_p[h]
                            if i == 0:
                                nc.tensor.matmul(
                                    S, kt_sb[:, 0:128], qt_sb[:, h, 0:512],
                                    start=True, stop=True,
                                )
                                nc.scalar.activation(
                                    pt0[:, 0:512], S, ACT_EXP,
                                    bias=ebias_sb, scale=SCALE,
                                )
                                nc.gpsimd.tensor_mul(
                                    pt0[:, 0:128], pt0[:, 0:128], tri_sb
                                )
                            elif i == 1:
                                nc.tensor.matmul(
                                    S[:, 0:384], kt_sb[:, 128:256],
                                    qt_sb[:, h, 128:512],
                                    start=True, stop=True,
                                )
                                nc.scalar.activation(
                                    pt0[:, 512:896], S[:, 0:384], ACT_EXP,
                                    bias=ebias_sb, scale=SCALE,
                                )
                                nc.gpsimd.tensor_mul(
                                    pt0[:, 512:640], pt0[:, 512:640], tri_sb
                                )
                            else:
                                nc.tensor.matmul(
                                    S[:, 0:256], kt_sb[:, 256:384],
                                    qt_sb[:, h, 256:512],
                                    start=True, stop=True,
                                )
                                nc.tensor.matmul(
                                    S[:, 256:384], kt_sb[:, 384:512],
                                    qt_sb[:, h, 384:512],
                                    start=True, stop=True,
                                )
                                nc.scalar.activation(
                                    pt0[:, 896:1280], S[:, 0:384], ACT_EXP,
                                    bias=ebias_sb, scale=SCALE,
                                )
                                nc.gpsimd.tensor_mul(
                                    pt0[:, 896:1024], pt0[:, 896:1024], tri_sb
                                )
                                nc.gpsimd.tensor_mul(
                                    pt0[:, 1152:1280], pt0[:, 1152:1280], tri_sb
                                )
                        return go

                    for h in range(HQ):
                        for i in range(3):
                            a0_pieces.append(a0_score_piece(h, i))

                def acc(i):
                    if i < 4:
                        return aq[i // 2][:, 512 * (i % 2) : 512 * (i % 2 + 1)]
                    return ak if i == 4 else av

                for o2 in range(16):
                    rtt2 = rpool.tile([128, 2, 512], F16, tag="rt", bufs=7)
                    if c == 0 and o2 == 0:
                        # singles: first matmul's dependency is ~256KB of
                        # DMA, not ~1MB
                        nc.sync.dma_start(rtt2[:, 0, :], rt3[:, 0, cs])
                        nc.sync.dma_start(wq_sb[:, 0, :], wq3[:, 0, :])
                        nc.sync.dma_start(wk_sb[:, 0:1, :], wk3[:, 0:1, :])
                        nc.sync.dma_start(wv_sb[:, 0:1, :], wv3[:, 0:1, :])
                        nc.sync.dma_start(rtt2[:, 1, :], rt3[:, 1, cs])
                        nc.sync.dma_start(wq_sb[:, 1, :], wq3[:, 1, :])
                        nc.sync.dma_start(wk_sb[:, 1:4, :], wk3[:, 1:4, :])
                        nc.sync.dma_start(wv_sb[:, 1:4, :], wv3[:, 1:4, :])
                    else:
                        nc.sync.dma_start(rtt2, rt3[:, 2 * o2 : 2 * o2 + 2, cs])
                        if c == 0 and o2 == 12:
                            # rope tables + mask, issued late in chunk 0 where
                            # the weight streams have finished: off both the
                            # chunk-0 and the chunk-1 rt critical paths
                            load_consts()
                            nc.sync.dma_start(cos_sb, cosT[:, :])
                            nc.sync.dma_start(sin_sb, sinM[:, :])
                        if c == 0:
                            o = 2 * o2
                            nc.sync.dma_start(
                                wq_sb[:, o : o + 2, :], wq3[:, o : o + 2, :]
                            )
                            if o % 4 == 0:
                                nc.sync.dma_start(
                                    wk_sb[:, o : o + 4, :], wk3[:, o : o + 4, :]
                                )
                                nc.sync.dma_start(
                                    wv_sb[:, o : o + 4, :], wv3[:, o : o + 4, :]
                                )
                    for oo in range(2):
                        o = 2 * o2 + oo
                        rtt = rtt2[:, oo, :]
                        st, sp = (o == 0), (o == 31)
                        for h in range(HQ):
                            nc.tensor.matmul(
                                acc(h), wq_sb[:, o, 128 * h : 128 * (h + 1)], rtt,
                                start=st, stop=sp,
                            )
                        nc.tensor.matmul(
                            acc(4), wk_sb[:, o, :], rtt, start=st, stop=sp
                        )
                        nc.tensor.matmul(
                            acc(5), wv_sb[:, o, :], rtt, start=st, stop=sp
                        )
                        if o in (8, 12, 16, 20):
                            finish_norm()  # previous attn chunk's norms
                        if o in (6, 10, 14, 18, 22):
                            flush_rope()  # pending rope tiles
                        if a0_pieces and o % 2 == 0 and 4 <= o <= 26:
                            a0_pieces.pop(0)()
                        if c == 2 and o in (8, 14, 20, 26):
                            hh = (o - 8) // 6
                            nc.sync.dma_start(wo_sb[:, hh, :], wo3[:, hh, :])
                # drain PSUM fast: one cast per accumulator (split
                # scalar/vector), then rope runs on fp16 SBUF tiles
                xq = []
                for idx in range(5):
                    x = spool.tile([128, 512], F16, tag=f"x{idx}")
                    if idx == 0 and c != 3:
                        nc.scalar.activation(x, acc(idx), ACT_COPY)
                    else:
                        # c==3: keep the scalar queue empty -- the merged
                        # phase's attention exps follow immediately
                        nc.vector.tensor_copy(x, acc(idx))
                    xq.append(x)
                vt = spool.tile([128, 512], F16, tag="vt")

                def rope_tile(idx, x=None):
                    def go(x=x):
                        xs = spool.tile([128, 512], F16, tag="xs", name=f"xs{c}_{idx}")
                        nc.vector.tensor_copy(xs[0:64, :], x[64:128, :])
                        nc.vector.tensor_copy(xs[64:128, :], x[0:64, :])
                        t1 = spool.tile([128, 512], F16, tag="t1", name=f"t1{c}_{idx}")
                        nc.vector.tensor_mul(t1, x, cos_sb[:, cs])
                        nc.vector.tensor_mul(xs, xs, sin_sb[:, cs])
                        dst = qt_sb[:, idx, cs] if idx < HQ else kt_sb[:, cs]
                        nc.vector.tensor_add(dst, t1, xs)
                    return go

                for idx in range(5):
                    rope_pend.append(rope_tile(idx, xq[idx]))
                # drain the v accumulator (split scalar/vector, frees its
                # PSUM bank), then V^T -> V via DMA-xbar transposes on the
                # otherwise-idle DMA engines. No PE instruction sits at the
                # phase boundary, so the next phase's matmuls never stall
                # behind the vt drain.
                if c != 3:
                    nc.scalar.activation(vt[:, 0:256], acc(5)[:, 0:256], ACT_COPY)
                else:
                    nc.vector.tensor_copy(vt[:, 0:256], acc(5)[:, 0:256])
                nc.vector.tensor_copy(vt[:, 256:512], acc(5)[:, 256:512])
                for s in range(4):
                    nc.sync.dma_start_transpose(
                        v_sb[:, 4 * c + s, :], vt[:, 128 * s : 128 * (s + 1)]
                    )
                if c == 2:
                    # attention chunk 0's ctx/den tail: p is exp'd+masked in
                    # SBUF, V chunk 0 landed two phases ago -- streams with
                    # no stalls into the banks the k/v drains just freed.
                    a0_geom = ((0, 512), (512, 384), (896, 256), (1152, 128))
                    for h in range(HQ):
                        pt0 = a0_p[h]
                        a0ctx = ps.tile(
                            [128, 512], F32, tag="ps", bufs=4, name=f"a0ctx{h}"
                        )
                        a0den = ps.tile(
                            [128, 512], F32, tag="ps", bufs=4, name=f"a0den{h}"
                        )
                        for r, (pc, F) in enumerate(a0_geom):
                            nc.tensor.matmul(
                                a0ctx[:, 512 - F : 512], v_sb[:, r, :],
                                pt0[:, pc : pc + F],
                                start=(r == 0), stop=(r == 3),
                            )
                            nc.tensor.matmul(
                                a0den[:, 512 - F : 512], ones_sb,
                                pt0[:, pc : pc + F],
                                start=(r == 0), stop=(r == 3),
                            )
                        den16 = spool.tile(
                            [128, 512], F16, tag="den16", bufs=6, name=f"a0d16_{h}"
                        )
                        nc.vector.tensor_copy(den16, a0den)
                        cxu = spool.tile(
                            [128, 512], F16, tag="cxu", bufs=6, name=f"a0cxu{h}"
                        )
                        nc.vector.tensor_copy(cxu, a0ctx)
                        pend.append((0, h, cxu, den16))

            def attn_units(c, lagp, ps_diag=False):
                """Incremental emitter for attention chunk c: a generator
                yielding after each (scores+exp | ctx+den) pipeline step.
                Globally pipelined across heads: unit stream is
                [(h,t) for h in heads for t in units-of-head]."""
                npair = 2 * c  # full pairs per head
                nunits = npair + 2  # + two diagonal pairs
                nj = 4 * (c + 1)
                state = {}  # h -> (ctx_ps, den_ps)
                p_tiles = {}
                pa_tiles = {}

                def emit_scores(h, t):
                    # standalone chunk 0 has no proj competing for "ps", so
                    # its second (narrow) diagonal pair allocates there --
                    # doubling the score-buffer depth of the exp pipeline.
                    if ps_diag and t == npair + 1:
                        s_ps = ps.tile(
                            [128, 512], F32, tag="ps", bufs=4, name=f"s{c}_{h}_{t}"
                        )
                    else:
                        s_ps = ps.tile(
                            [128, 1024], F32, tag="psq", bufs=2, name=f"s{c}_{h}_{t}"
                        )
                    p = ppool.tile([128, 1024], F16, tag="pt", name=f"p{c}_{h}_{t}")
                    if t < npair:  # full pair
                        qs_full = qt_sb[:, h, 512 * c : 512 * (c + 1)]
                        for u in range(2):
                            j = 2 * t + u
                            nc.tensor.matmul(
                                s_ps[:, 512 * u : 512 * (u + 1)],
                                kt_sb[:, 128 * j : 128 * (j + 1)],
                                qs_full,
                                start=True,
                                stop=True,
                            )
                        nc.scalar.activation(
                            p, s_ps, ACT_EXP, bias=ebias_sb, scale=SCALE
                        )
                        pa = spool.tile(
                            [128, 512], F16, tag="pa", bufs=4, name=f"pa{c}_{h}_{t}"
                        )
                        nc.vector.tensor_add(pa, p[:, 0:512], p[:, 512:1024])
                        if t % 2 == 1:
                            paq = spool.tile(
                                [128, 512], F16, tag="paq", bufs=3,
                                name=f"paq{c}_{h}_{t}",
                            )
                            nc.vector.tensor_add(paq, pa_tiles.pop((h, t - 1)), pa)
                            pa_tiles[(h, t)] = paq
                        else:
                            pa_tiles[(h, t)] = pa
                    else:  # diagonal pair
                        d = t - npair
                        offs = (0, 512) if d == 0 else (0, 256)
                        for u in range(2):
                            r = 2 * d + u
                            j = 4 * c + r
                            F = 512 - 128 * r
                            nc.tensor.matmul(
                                s_ps[:, offs[u] : offs[u] + F],
                                kt_sb[:, 128 * j : 128 * (j + 1)],
                                qt_sb[:, h, 512 * (c + 1) - F : 512 * (c + 1)],
                                start=True,
                                stop=True,
                            )
                        W = 896 if d == 0 else 384
                        nc.scalar.activation(
                            p[:, 0:W], s_ps[:, 0:W], ACT_EXP,
                            bias=ebias_sb, scale=SCALE,
                        )
                        for u in range(2):
                            off = offs[u]
                            nc.gpsimd.tensor_mul(
                                p[:, off : off + 128], p[:, off : off + 128], tri_sb
                            )
                    p_tiles[(h, t)] = p

                def emit_ctxden(h, t):
                    if t == 0:
                        ctx_ps = ps.tile(
                            [128, 512], F32, tag="ps", bufs=4, name=f"ctx{c}_{h}"
                        )
                        den_ps = ps.tile(
                            [128, 512], F32, tag="ps", bufs=4, name=f"den{c}_{h}"
                        )
                        state[h] = (ctx_ps, den_ps)
                    ctx_ps, den_ps = state[h]
                    p = p_tiles.pop((h, t))
                    if t < npair:
                        for u in range(2):
                            j = 2 * t + u
                            nc.tensor.matmul(
                                ctx_ps,
                                v_sb[:, j, :],
                                p[:, 512 * u : 512 * (u + 1)],
                                start=(j == 0),
                                stop=False,
                            )
                        if t % 2 == 1:
                            nc.tensor.matmul(
                                den_ps,
                                ones_sb,
                                pa_tiles.pop((h, t)),
                                start=(t == 1),
                                stop=False,
                            )
                    else:
                        d = t - npair
                        offs = (0, 512) if d == 0 else (0, 256)
                        for u in range(2):
                            r = 2 * d + u
                            j = 4 * c + r
                            F = 512 - 128 * r
                            nc.tensor.matmul(
                                ctx_ps[:, 512 - F : 512],
                                v_sb[:, j, :],
                                p[:, offs[u] : offs[u] + F],
                                start=(j == 0),
                                stop=(j == nj - 1),
                            )
                            nc.tensor.matmul(
                                den_ps[:, 512 - F : 512],
                                ones_sb,
                                p[:, offs[u] : offs[u] + F],
                                start=(c == 0 and r == 0),
                                stop=(r == 3),
                            )
                    if t == nunits - 1:
                        # drain den and ctx to SBUF, freeing both banks.
                        # den in fp16 is plenty: den in [1e-3, 250], so
                        # ln(den) picks up <~2e-3 absolute -> <0.2% on rec.
                        den16 = spool.tile(
                            [128, 512], F16, tag="den16", bufs=6, name=f"d16_{c}_{h}"
                        )
                        nc.vector.tensor_copy(den16, den_ps)
                        cxu = spool.tile(
                            [128, 512], F16, tag="cxu", bufs=6, name=f"cxu{c}_{h}"
                        )
                        nc.vector.tensor_copy(cxu, ctx_ps)
                        pend.append((c, h, cxu, den16))
                        del state[h]

                units = [(h, t) for h in range(HQ) for t in range(nunits)]
                n = len(units)
                for i in range(n + lagp):
                    if i < n:
                        emit_scores(*units[i])
                    if i == 0:
                        flush_rope()
                    j = i - lagp
                    if j >= 0:
                        emit_ctxden(*units[j])
                    yield

            def run_gen(gen):
                for _ in gen:
                    pass

            def wo_blocks(c, last=False, vec_drains=False):
                """Incremental emitter for Wo chunk c: 4 sections x 4 blocks;
                each block = 2 single-bank PSUM tiles, 8 matmuls (4 heads x
                2 col-groups), 2 drains. Yields after each block."""
                for s in range(4):
                    tq = 4 * c + s
                    flush_rope()
                    finish_norm()
                    for bi in range(4):
                        if bi == 2:
                            finish_norm()
                        half, grp = bi // 2, bi % 2
                        if grp == 0:
                            ob = opool.tile(
                                [128, 4, 512], F16, tag="ob", name=f"ob{tq}_{half}"
                            )
                        pw = [
                            ps.tile(
                                [128, 512], F32, tag="ps", bufs=4,
                                name=f"wops{tq}_{bi}_{i}",
                            )
                            for i in range(2)
                        ]
                        for h in range(HQ):
                            lhsT = cx_sb[:, h, 128 * tq : 128 * (tq + 1)]
                            for i in range(2):
                                n = 4 * half + 2 * grp + i
                                nc.tensor.matmul(
                                    pw[i],
                                    lhsT,
                                    wo_sb[:, h, 512 * n : 512 * (n + 1)],
                                    start=(h == 0),
                                    stop=(h == HQ - 1),
                                )
                        for i in range(2):
                            n = 4 * half + 2 * grp + i
                            no = 2 * grp + i
                            if i == 0 or vec_drains:
                                # in merged phases the scalar engine is busy
                                # with attention exps; drain on vector only
                                nc.vector.tensor_copy(ob[:, no, :], pw[i])
                            else:
                                nc.scalar.activation(ob[:, no, :], pw[i], ACT_COPY)
                        if last and s == 3:
                            # final section: store each block's quarter as
                            # soon as its drains are emitted, so the kernel
                            # tail is one drain + one short store
                            nn = 4 * half + 2 * grp
                            nc.sync.dma_start(
                                outp[
                                    128 * tq : 128 * (tq + 1),
                                    512 * nn : 512 * (nn + 2),
                                ],
                                ob[:, 2 * grp : 2 * grp + 2, :],
                            )
                        elif grp == 1:
                            # half-row output DMA: starts the store while the
                            # other half still computes
                            nc.sync.dma_start(
                                outp[
                                    128 * tq : 128 * (tq + 1),
                                    2048 * half : 2048 * (half + 1),
                                ],
                                ob[:, :, :],
                            )
                        yield

            def merged(wo_c, attn_cs, attn_share):
                """Interleave one Wo chunk (16 blocks) with the chained unit
                streams of one or more attention chunks; attn_share units are
                emitted after each wo block (list of 16 ints)."""
                wg = wo_blocks(wo_c)
                ags = [attn_units(c, lagp=2) for c in attn_cs]

                def steps():
                    for ag in ags:
                        yield from ag

                ag = steps()
                # prime the exp pipeline: two units of scores ahead of the
                # first wo block so the scalar engine starts early
                for _ in range(2):
                    next(ag, None)
                for k in attn_share:
                    next(wg)
                    for _ in range(k):
                        next(ag, None)
                run_gen(ag)
                run_gen(wg)

            # emission order ~ per-engine execution order
            proj_chunk(0)
            proj_chunk(1)
            proj_chunk(2)
            proj_chunk(3)
            # attn1+attn2 (18+26 pipeline steps) into wo0, attn3 (34) into
            # wo1: their exps run against the Wo phases' idle scalar time
            merged(0, (1, 2), [3] * 12 + [2] * 4)
            merged(1, (3,), [2] * 16)
            run_gen(wo_blocks(2))
            run_gen(wo_blocks(3, last=True))
            while pend:
                finish_norm()

    bass_rust.generate_event_semaphores(nc)
    return nc


_NC = None


def _get_nc():
    global _NC
    if _NC is None:
        _NC = _build_nc()
    return _NC


def _host_inputs(resid, Wq, Wk, Wv, Wo):
    f16 = np.float16
    r2 = np.asarray(resid, dtype=np.float32).reshape(T, D_MODEL)
    rt = np.ascontiguousarray(r2.T).astype(f16)  # [D, T]
    cosT, sinM = _rope_tables()
    f = np.arange(128)[None, :]
    p = np.arange(128)[:, None]
    tri = (p <= f).astype(f16)  # [128, 128]
    Wq = np.asarray(Wq, np.float32)
    Wk = np.asarray(Wk, np.float32)
    Wv = np.asarray(Wv, np.float32)
    Wo = np.asarray(Wo, np.float32)
    in_maps = []
    for i in range(NCORES):
        in_maps.append(
            {
                "rt": rt,
                "wq": np.ascontiguousarray(Wq[:, 512 * i : 512 * (i + 1)]).astype(f16),
                "wk": np.ascontiguousarray(Wk[:, 128 * i : 128 * (i + 1)]).astype(f16),
                "wv": np.ascontiguousarray(Wv[:, 128 * i : 128 * (i + 1)]).astype(f16),
                "wo": np.ascontiguousarray(Wo[512 * i : 512 * (i + 1), :]).astype(f16),
                "cosT": cosT,
                "sinM": sinM,
                "tri": tri,
            }
        )
    return in_maps


def run(resid, Wq, Wk, Wv, Wo, **spmd_kwargs):
    in_maps = _host_inputs(resid, Wq, Wk, Wv, Wo)
    nc = _get_nc()
    res = run_bass_kernel_spmd(nc, in_maps, core_ids=list(range(NCORES)), **spmd_kwargs)
    out = np.zeros((T, D_MODEL), np.float32)
    for rmap in res.results:
        out += rmap["outp"].astype(np.float32)
    return out.reshape(1, T, D_MODEL), res


def kernel(resid, Wq, Wk, Wv, Wo):
    # warm-up execution: activation tables and DMA rings are only guaranteed
    # after one execution has cycled them; the second execution is the
    # validated-correct path.
    run(resid, Wq, Wk, Wv, Wo)
    out, _ = run(resid, Wq, Wk, Wv, Wo)
    return out
